# revision 1
# baseline (speedup 1.0000x reference)
"""Trainium2 Bass kernel for nn_DSEBlock: FEA (multi-scale bilinear edge) +
DoG (difference-of-gaussians depthwise) + 1x1 mixer, data-parallel over batch
on 8 NeuronCores.

Decomposition (validated vs reference to ~1e-6 in fp64):
  y = dec + skip
  per scale s in {.25,.5,.75}:  r_s = (Uh Dh) x (Uw Dw) y ; d_s = y - r_s
  w_edge = (2/3)(max_s|d_s| - min_s|d_s|)
  dog    = n1*G1 - n2*G2, G_i = separable [t,1,t] x [t,1,t] conv (zero pad)
  out    = mixer @ (3y + w_fea*w_edge + dog) + skip

Engine mapping:
  - back half in C-layout [c(128part), h, w]; FEA front in W-layout
    [w(96part), h(planar), c] reached via PE transposes.
  - W-axis resize = 96x96 matrix on PE.  H-axis: down on DVE; the up-lerp,
    the y-subtract are fused into residue-scaled accumulating PE matmuls
    (psum = y - r directly, in h-residue-planar order).
  - |d| on ACT; running max/min on DVE; wedge transposed back on PE with
    w_fea' folded into the PSUM->SBUF copy.
  - DoG separable on DVE with gaussian-2 chain on GPSIMD.  3y folds into the
    mixer as extra accumulating matmuls with weights 3*M.
"""
import functools

import ml_dtypes
import numpy as np

import concourse.bass as bass
import concourse.mybir as mybir
import concourse.tile as tile
from concourse import bacc
from concourse.bass import ts
from concourse.bass_utils import run_bass_kernel_spmd
from concourse.masks import make_identity

F32 = mybir.dt.float32
BF16 = mybir.dt.bfloat16
AL = mybir.AluOpType
AF = mybir.ActivationFunctionType

B, C, H, W = 16, 256, 96, 96
NCORES = 8
BPC = B // NCORES
SCALES = [0.25, 0.5, 0.75]
NS = [24, 48, 72]
HW = H * W


def _sl(start, step, cnt):
    return slice(start, start + step * (cnt - 1) + 1, step)


# ---------------- host-side resize specs ----------------
def _resize_matrix(n_in, n_out):
    R = np.zeros((n_out, n_in), dtype=np.float64)
    scale = n_in / n_out
    for j in range(n_out):
        x = (j + 0.5) * scale - 0.5
        i0 = int(np.floor(x))
        f = x - i0
        R[j, min(max(i0, 0), n_in - 1)] += 1.0 - f
        R[j, min(max(i0 + 1, 0), n_in - 1)] += f
    return R


def _down_ops(s):
    if s == 0.25:
        return [("avg", (0, 1, 24), (1, 4), (2, 4))]
    if s == 0.5:
        return [("avg", (0, 1, 48), (0, 2), (1, 2))]
    assert s == 0.75
    return [
        ("lerp", (r, 3, 24), (i0, 4), (i0 + 1, 4), f)
        for r, (i0, f) in enumerate([(0, 1 / 6), (1, 1 / 2), (2, 5 / 6)])
    ]


def _up_ops(s):
    """Interior lerp runs per residue r=j%4 plus edge-clamp copies.

    returns (runs, copies): runs[r] = (m0, cnt, a0, S, f) covering
    j = 4m+r, m in [m0, m0+cnt); z indices a0 + S*(m-m0) (+1).
    copies: list of (j, src)."""
    ns = int(H * s)
    scale = ns / H
    S = {0.25: 1, 0.5: 2, 0.75: 3}[s]
    groups, copies = {}, []
    for j in range(H):
        x = (j + 0.5) * scale - 0.5
        i0 = int(np.floor(x))
        f = x - i0
        if i0 < 0:
            copies.append((j, 0))
            continue
        if i0 + 1 > ns - 1:
            copies.append((j, ns - 1))
            continue
        groups.setdefault(j % 4, []).append((j // 4, i0, f))
    runs = {}
    for r, items in groups.items():
        items.sort()
        ms = [m for m, _, _ in items]
        assert ms == list(range(ms[0], ms[-1] + 1))
        fs = {round(f, 9) for _, _, f in items}
        assert len(fs) == 1
        runs[r] = (ms[0], len(ms), items[0][1], S, items[0][2])
    return runs, copies


# ---------------- program (input-independent; cached) ----------------
@functools.lru_cache(maxsize=1)
def _build():
    nc = bacc.Bacc("TRN2", target_bir_lowering=False, debug=False)
    dec_d = nc.dram_tensor("dec", [BPC, C, H, W], F32, kind="ExternalInput")
    skip_d = nc.dram_tensor("skip", [BPC, C, H, W], F32, kind="ExternalInput")
    aw_d = nc.dram_tensor("aw", [96, 27, 96], BF16, kind="ExternalInput")
    mw_d = nc.dram_tensor("mw", [128, 8, 128], BF16, kind="ExternalInput")
    coef_d = nc.dram_tensor("coef", [128, 12], F32, kind="ExternalInput")
    out_d = nc.dram_tensor("out", [BPC, C, H, W], F32, kind="ExternalOutput")

    dn_ops = [_down_ops(s) for s in SCALES]
    up_runs = [_up_ops(s)[0] for s in SCALES]
    up_cp = [_up_ops(s)[1] for s in SCALES]
    # clamp slots by planar position p = r*24 + m
    cp_by_p = [
        {(j % 4) * 24 + j // 4: (j, src) for j, src in up_cp[si]} for si in range(3)
    ]

    with tile.TileContext(nc) as tc:
        with (
            tc.tile_pool(name="const", bufs=1) as pconst,
            tc.tile_pool(name="stage", bufs=2) as pstage,
            tc.tile_pool(name="py", bufs=1) as py,
            tc.tile_pool(name="pyw", bufs=1) as pyw,
            tc.tile_pool(name="pmm", bufs=1) as pmm,
            tc.tile_pool(name="pwk", bufs=2) as pwk,
            tc.tile_pool(name="pdl", bufs=1) as pdl,
            tc.tile_pool(name="pwc", bufs=1) as pwc,
            tc.tile_pool(name="pdga", bufs=1) as pdga,
            tc.tile_pool(name="pdgb", bufs=2) as pdgb,
            tc.tile_pool(name="pp4", bufs=2) as pp4,
            tc.tile_pool(name="ps_ytr", bufs=2, space="PSUM") as ps_ytr,
            tc.tile_pool(name="ps_d", bufs=2, space="PSUM") as ps_d,
            tc.tile_pool(name="ps_wed", bufs=2, space="PSUM") as ps_wed,
            tc.tile_pool(name="ps_mix", bufs=2, space="PSUM") as ps_mix,
            tc.tile_pool(name="dram", bufs=2, space="DRAM") as pdram,
        ):
            aw_sb = pconst.tile([96, 27, 96], BF16)
            nc.sync.dma_start(out=aw_sb[:], in_=aw_d[:])
            mw_sb = pconst.tile([128, 8, 128], BF16)
            nc.sync.dma_start(out=mw_sb[:], in_=mw_d[:])
            coef_sb = pconst.tile([128, 12], F32)
            nc.sync.dma_start(out=coef_sb[:], in_=coef_d[:])
            ident = pconst.tile([128, 128], BF16)
            make_identity(nc, ident[:])

            def cf(cb, j, psl=slice(0, 128)):
                return coef_sb[psl, cb * 6 + j : cb * 6 + j + 1]

            def emit_y(s, cb):
                csl_d = slice(cb * 128, (cb + 1) * 128)
                y = py.tile([128, H, W], BF16, tag=f"y{cb}", name=f"y_{s}_{cb}")
                for st in range(16):
                    rsl = slice(st * 6, st * 6 + 6)
                    td = pstage.tile([128, 6, W], F32, tag="std", name=f"td{s}{cb}{st}")
                    nc.sync.dma_start(out=td[:], in_=dec_d[s, csl_d, rsl])
                    tk = pstage.tile([128, 6, W], F32, tag="stk", name=f"tk{s}{cb}{st}")
                    nc.sync.dma_start(out=tk[:], in_=skip_d[s, csl_d, rsl])
                    nc.vector.tensor_add(out=y[:, rsl, :], in0=td[:], in1=tk[:])
                return y

            def emit_hdown(si, yw, cfs, s, cb, hf):
                hd = pwk.tile([96, H, 64], BF16, tag="wk", name=f"hd{s}{cb}{hf}{si}")
                for op in dn_ops[si]:
                    if op[0] == "avg":
                        (o0, ostep, cnt), (a0, astep), (b0, bstep) = op[1:]
                        nc.vector.tensor_add(
                            out=hd[:, _sl(o0, ostep, cnt), :],
                            in0=yw[:, _sl(a0, astep, cnt), cfs],
                            in1=yw[:, _sl(b0, bstep, cnt), cfs],
                        )
                    else:
                        (o0, ostep, cnt), (a0, astep), (b0, bstep), f = op[1:]
                        dl = pdl.tile([96, 24, 64], BF16, tag="dl")
                        nc.vector.tensor_sub(
                            out=dl[:],
                            in0=yw[:, _sl(b0, bstep, cnt), cfs],
                            in1=yw[:, _sl(a0, astep, cnt), cfs],
                        )
                        nc.vector.scalar_tensor_tensor(
                            out=hd[:, _sl(o0, ostep, cnt), :],
                            in0=dl[:],
                            scalar=float(f),
                            in1=yw[:, _sl(a0, astep, cnt), cfs],
                            op0=AL.mult,
                            op1=AL.add,
                        )
                return hd

            def emit_dbanks(si, hd, yw, cfs, mx, abs_dst):
                for b in range(12):
                    r = (8 * b) // 24
                    mlo = (8 * b) % 24
                    pd = ps_d.tile([96, 8, 64], F32, tag="pd")
                    m0, cnt, a0, S0, f = up_runs[si][r]
                    ilo, ihi = max(mlo, m0), min(mlo + 8, m0 + cnt)
                    # single accumulation group per bank: y first (start),
                    # then all taps accumulate, last one stops.
                    mms = []
                    if ihi > ilo:
                        n = ihi - ilo
                        av = a0 + S0 * (ilo - m0)
                        sl_o = pd[:, ilo - mlo : ihi - mlo, :]
                        mms.append((sl_o, si * 9 + 2 * r, hd[:, _sl(av, S0, n), :]))
                        mms.append((sl_o, si * 9 + 2 * r + 1, hd[:, _sl(av + 1, S0, n), :]))
                    for mm in range(mlo, mlo + 8):
                        pp = r * 24 + mm
                        if pp in cp_by_p[si]:
                            _, src = cp_by_p[si][pp]
                            mms.append((pd[:, mm - mlo, :], si * 9 + 8, hd[:, src, :]))
                    nc.tensor.matmul(
                        pd[:],
                        lhsT=ident[0:96, 0:96],
                        rhs=yw[:, _sl(4 * mlo + r, 4, 8), cfs],
                        start=True, stop=False,
                    )
                    for i, (out_ap, vi, rhs_ap) in enumerate(mms):
                        nc.tensor.matmul(
                            out_ap,
                            lhsT=aw_sb[:, vi, :],
                            rhs=rhs_ap,
                            start=False,
                            stop=(i == len(mms) - 1),
                        )
                    if abs_dst is None:
                        nc.scalar.activation(mx[:, 8 * b : 8 * b + 8, cfs], pd[:], AF.Abs)
                    else:
                        nc.scalar.activation(abs_dst[:, 8 * b : 8 * b + 8, :], pd[:], AF.Abs)

            def emit_front(s, cb, y):
                yw = pyw.tile([96, H, 128], BF16, tag="yw", name=f"yw_{s}_{cb}")
                for hb in range(24):
                    pt = ps_ytr.tile([96, 4, 128], BF16, tag="ptr", name=f"pt{s}{cb}{hb}")
                    for k in range(4):
                        nc.tensor.transpose(pt[:, k, :], y[:, hb * 4 + k, :], ident[:])
                    nc.scalar.copy(yw[:, hb * 4 : hb * 4 + 4, :], pt[:])
                mx = pmm.tile([96, H, 128], BF16, tag="mx", name=f"mx{s}{cb}")
                mn = pmm.tile([96, H, 128], BF16, tag="mn", name=f"mn{s}{cb}")
                for hf in range(2):
                    cfs = slice(hf * 64, hf * 64 + 64)
                    for si in range(3):
                        hd = emit_hdown(si, yw, cfs, s, cb, hf)
                        abs_dst = (
                            None if si == 0
                            else pwk.tile([96, H, 64], BF16, tag="wk", name=f"ab{s}{cb}{hf}{si}")
                        )
                        emit_dbanks(si, hd, yw, cfs, mx, abs_dst)
                        if si == 0:
                            nc.gpsimd.tensor_copy(out=mn[:, :, cfs], in_=mx[:, :, cfs])
                        else:
                            nc.vector.tensor_tensor(
                                out=mx[:, :, cfs], in0=mx[:, :, cfs], in1=abs_dst[:], op=AL.max
                            )
                            nc.vector.tensor_tensor(
                                out=mn[:, :, cfs], in0=mn[:, :, cfs], in1=abs_dst[:], op=AL.min
                            )
                nc.vector.tensor_sub(out=mx[:], in0=mx[:], in1=mn[:])
                wcon = pwc.tile([128, H, W], BF16, tag="wcon", name=f"wc_{s}_{cb}")
                for q in range(24):
                    pw = ps_wed.tile([128, 4, 96], BF16, tag="pwed")
                    for i in range(4):
                        nc.tensor.transpose(pw[:, i, :], mx[:, 4 * q + i, :], ident[0:96, 0:96])
                    p0 = 4 * q
                    r, m = p0 // 24, p0 % 24
                    nc.scalar.activation(
                        wcon[:, _sl(4 * m + r, 4, 4), :], pw[:], AF.Copy, scale=cf(cb, 0)
                    )
                return wcon

            def emit_dog(s, cb, y, wcon, zd):
                for st in range(12):
                    h0 = st * 8
                    r0, r1 = max(h0 - 1, 0), min(h0 + 9, H)
                    nr = r1 - r0
                    ctr = slice(h0 - r0, h0 - r0 + 8)
                    at = pdga.tile([128, 10, W], BF16, tag="at")
                    nc.vector.tensor_add(
                        out=at[:, :nr, 1:95], in0=y[:, r0:r1, 0:94], in1=y[:, r0:r1, 2:96]
                    )
                    nc.vector.tensor_copy(out=at[:, :nr, 0], in_=y[:, r0:r1, 1])
                    nc.vector.tensor_copy(out=at[:, :nr, 95], in_=y[:, r0:r1, 94])
                    pg = {}
                    for gi in (1, 2):
                        pt_ = pdga.tile([128, 10, W], BF16, tag=f"pg{gi}")
                        nc.vector.scalar_tensor_tensor(
                            out=pt_[:, :nr, :], in0=at[:, :nr, :], scalar=cf(cb, gi),
                            in1=y[:, r0:r1, :], op0=AL.mult, op1=AL.add,
                        )
                        pg[gi] = pt_
                    zt = None
                    for gi in (1, 2):
                        pt_ = pg[gi]
                        bt = pdgb.tile([128, 8, W], BF16, tag="bt")
                        g0, g1 = max(h0, 1), min(h0 + 8, 95)
                        nc.gpsimd.tensor_tensor(
                            out=bt[:, g0 - h0 : g1 - h0, :],
                            in0=pt_[:, g0 - 1 - r0 : g1 - 1 - r0, :],
                            in1=pt_[:, g0 + 1 - r0 : g1 + 1 - r0, :],
                            op=AL.add,
                        )
                        if h0 == 0:
                            nc.vector.tensor_copy(out=bt[:, 0, :], in_=pt_[:, 1, :])
                        if h0 + 8 == H:
                            nc.vector.tensor_copy(out=bt[:, 7, :], in_=pt_[:, 94 - r0, :])
                        gt = pdgb.tile([128, 8, W], BF16, tag=f"gt{gi}")
                        nc.vector.scalar_tensor_tensor(
                            out=gt[:], in0=bt[:], scalar=cf(cb, gi),
                            in1=pt_[:, ctr, :], op0=AL.mult, op1=AL.add,
                        )
                        ztn = pdgb.tile([128, 8, W], BF16, tag="zt")
                        nc.vector.scalar_tensor_tensor(
                            out=ztn[:], in0=gt[:], scalar=cf(cb, 2 + gi),
                            in1=wcon[:, h0 : h0 + 8, :] if gi == 1 else zt[:],
                            op0=AL.mult, op1=AL.add,
                        )
                        zt = ztn
                    zf = pdgb.tile([128, 8, W], BF16, tag="zt")
                    nc.vector.scalar_tensor_tensor(
                        out=zf[:], in0=y[:, h0 : h0 + 8, :], scalar=3.0,
                        in1=zt[:], op0=AL.mult, op1=AL.add,
                    )
                    nc.sync.dma_start(out=zd[:, h0 * W : (h0 + 8) * W], in_=zf[:])

            def emit_p4(s, ys, zds):
                yf = [t[:].rearrange("c h w -> c (h w)") for t in ys]
                for ob in range(2):
                    osl = slice(ob * 128, (ob + 1) * 128)
                    of = out_d[s, osl].rearrange("c h w -> c (h w)")
                    kf = skip_d[s, osl].rearrange("c h w -> c (h w)")
                    for ng in range(18):
                        za0 = pp4.tile([128, 512], BF16, tag="za0")
                        nc.sync.dma_start(out=za0[:], in_=zds[0][:, ts(ng, 512)])
                        za1 = pp4.tile([128, 512], BF16, tag="za1")
                        nc.sync.dma_start(out=za1[:], in_=zds[1][:, ts(ng, 512)])
                        sk = pp4.tile([128, 512], F32, tag="sk")
                        nc.sync.dma_start(out=sk[:], in_=kf[:, ts(ng, 512)])
                        ot = pp4.tile([128, 512], F32, tag="ot")
                        pmx = ps_mix.tile([128, 512], F32, tag="pmix")
                        nc.tensor.matmul(pmx[:], lhsT=mw_sb[:, ob, :], rhs=za0[:], start=True, stop=False)
                        nc.tensor.matmul(pmx[:], lhsT=mw_sb[:, 2 + ob, :], rhs=za1[:], start=False, stop=True)
                        nc.vector.tensor_add(out=ot[:], in0=pmx[:], in1=sk[:])
                        nc.sync.dma_start(out=of[:, ts(ng, 512)], in_=ot[:])

            for s in range(BPC):
                zds = [
                    pdram.tile([128, HW], BF16, tag=f"zd{cb}", name=f"zd{cb}_{s}")
                    for cb in range(2)
                ]
                ys = []
                for cb in range(2):
                    y = emit_y(s, cb)
                    ys.append(y)
                    wcon = emit_front(s, cb, y)
                    emit_dog(s, cb, y, wcon, zds[cb])
                emit_p4(s, ys, zds)
    nc.finalize()
    return nc


# ---------------- host entry ----------------
def _consts(w_fea, sigma1, sigma2, mixer_w):
    wf = (w_fea.reshape(C).astype(np.float64)) * (2.0 / 3.0)
    tn = []
    for sg in (sigma1, sigma2):
        sig = 2.0 / (1.0 + np.exp(-sg.reshape(C).astype(np.float64)))
        t = np.exp(-1.0 / (2.0 * sig**2))
        tn.append((t, (1.0 + 2.0 * t) ** -2))
    (t1, n1), (t2, n2) = tn
    coef = np.zeros((128, 12), dtype=np.float32)
    for cb in range(2):
        ch = slice(cb * 128, (cb + 1) * 128)
        coef[:, cb * 6 + 0] = wf[ch]
        coef[:, cb * 6 + 1] = t1[ch]
        coef[:, cb * 6 + 2] = t2[ch]
        coef[:, cb * 6 + 3] = n1[ch]
        coef[:, cb * 6 + 4] = -n2[ch]

    aw = np.zeros((96, 27, 96), dtype=np.float64)
    for si, s in enumerate(SCALES):
        ns = int(H * s)
        A = _resize_matrix(ns, H) @ _resize_matrix(H, ns)
        fold = 0.5 if s in (0.25, 0.5) else 1.0
        Af = fold * A  # (96h', ns-ish) acting along the W axis: [w', w]
        runs, _ = _up_ops(s)
        for r, (m0, cnt, a0, S0, f) in runs.items():
            aw[:, si * 9 + 2 * r, :] = (-(1.0 - f) * Af).T
            aw[:, si * 9 + 2 * r + 1, :] = (-f * Af).T
        aw[:, si * 9 + 8, :] = (-Af).T
    aw = aw.astype(ml_dtypes.bfloat16)

    M = mixer_w.reshape(C, C).astype(np.float64)
    mw = np.zeros((128, 8, 128), dtype=np.float64)
    for kc in range(2):
        for ob in range(2):
            blk = M[ob * 128 : (ob + 1) * 128, kc * 128 : (kc + 1) * 128].T
            mw[:, kc * 2 + ob, :] = blk
            mw[:, 4 + kc * 2 + ob, :] = 3.0 * blk
    mw = mw.astype(ml_dtypes.bfloat16)
    return aw, mw, coef


def kernel(skip, dec, w_fea, sigma1, sigma2, mixer_w, _trace=[False]):
    skip = np.ascontiguousarray(np.asarray(skip, dtype=np.float32))
    dec = np.ascontiguousarray(np.asarray(dec, dtype=np.float32))
    aw, mw, coef = _consts(
        np.asarray(w_fea), np.asarray(sigma1), np.asarray(sigma2), np.asarray(mixer_w)
    )
    nc = _build()
    in_maps = []
    for i in range(NCORES):
        in_maps.append(
            {
                "dec": dec[BPC * i : BPC * (i + 1)],
                "skip": skip[BPC * i : BPC * (i + 1)],
                "aw": aw,
                "mw": mw,
                "coef": coef,
            }
        )
    res = run_bass_kernel_spmd(nc, in_maps, core_ids=list(range(NCORES)), trace=_trace[0])
    kernel.last_result = res
    return np.concatenate([r["out"] for r in res.results], axis=0)


kernel.last_result = None



# revision 3
# speedup vs baseline: 1.3184x; 1.3184x over previous
"""Trainium2 Bass kernel v2 for nn_DSEBlock — engine-balanced redesign.

Decomposition (validated in mirror.py vs reference):
  y = dec + skip                         (DMA-accumulated, fp32, cast bf16)
  FEA: per scale si: psum = y - U_si(hd_si) x A_w,si  (PE, planar 4-row banks)
       e_si = |psum| (ACT abs) ; wedge = max3(e) - min3(e) (DVE/GP slab TT)
       wcon[c,h,w] = XBAR-DMA row transposes of wedge slabs
  DoG: u = S_w y (DVE), v = S_h y (GP), c1 = v + u (gpsimd accum-DMA),
       Q = S_h u (DVE)
  z   = (3+a)y + b*c1 + g*Q + wf'*wcon   (AFFINE_THEN_ADD chain + ACT)
  out = M z + skip                       (PE mixer, DVE psum+skip extract)

Engines: PE ~250us, DVE ~300, ACT ~280, GP ~240, DMA ~270 per core (est).
"""
import functools

import ml_dtypes
import numpy as np

import concourse.bass as bass
import concourse.mybir as mybir
import concourse.tile as tile
from concourse import bacc
from concourse.bass import ts
from concourse.bass_utils import run_bass_kernel_spmd
from concourse.dve_ops import AFFINE_THEN_ADD
from concourse.masks import make_identity

F32 = mybir.dt.float32
BF16 = mybir.dt.bfloat16
AL = mybir.AluOpType
AF = mybir.ActivationFunctionType

B, C, H, W = 16, 256, 96, 96
NCORES = 8
BPC = B // NCORES
SCALES = [0.25, 0.5, 0.75]
NS = [24, 48, 72]
HW = H * W


def _sl(start, step, cnt):
    return slice(start, start + step * (cnt - 1) + 1, step)


# ---------------- host-side resize structure ----------------
def _resize_matrix(n_in, n_out):
    A = np.zeros((n_out, n_in), dtype=np.float64)
    scale = n_in / n_out
    for j in range(n_out):
        x = (j + 0.5) * scale - 0.5
        i0 = int(np.floor(x))
        f = x - i0
        A[j, min(max(i0, 0), n_in - 1)] += 1.0 - f
        A[j, min(max(i0 + 1, 0), n_in - 1)] += f
    return A


def _up_runs(ns):
    scale = ns / H
    S = {24: 1, 48: 2, 72: 3}[ns]
    groups, copies = {}, []
    for j in range(H):
        x = (j + 0.5) * scale - 0.5
        i0 = int(np.floor(x))
        f = x - i0
        if i0 < 0:
            copies.append((j, 0))
            continue
        if i0 + 1 > ns - 1:
            copies.append((j, ns - 1))
            continue
        groups.setdefault(j % 4, []).append((j // 4, i0, f))
    runs = {}
    for r, items in groups.items():
        items.sort()
        ms = [m for m, _, _ in items]
        assert ms == list(range(ms[0], ms[-1] + 1))
        runs[r] = (ms[0], len(ms), items[0][1], S, items[0][2])
    return runs, copies


# aw slot map: per si: base = si*12: slots base+2r (tap a), base+2r+1 (tap b),
# base+8: clamp full, base+9: clamp half (si2 residue-1 sources)
NSLOT = 30


def _ywrows(yw, start, step, cnt):
    return yw[:, _sl(start, step, cnt), :]


def _si2_scale(a):
    return 0.5 if a % 3 == 1 else 1.0


UP = [_up_runs(ns) for ns in NS]


def _si2_plan():
    """B75 = U75@D75 folded tap plan: per r: interior run (m0, cnt,
    taps=[(c0, w)..]) with cols c0+3m, plus odd rows [(m, [(col, w)..])]."""
    B = _resize_matrix(72, 96) @ _resize_matrix(96, 72)
    plan = {}
    for r in range(4):
        rows = {}
        for m in range(24):
            j = 4 * m + r
            cols = np.nonzero(np.abs(B[j]) > 1e-12)[0]
            rows[m] = [(int(c), float(B[j, c])) for c in cols]
        # find the longest run of m where (cols - 3m, weights) identical
        def key(m):
            return tuple((c - 3 * m, round(w, 9)) for c, w in rows[m])
        best = None
        m = 0
        while m < 24:
            k = key(m)
            m2 = m
            while m2 + 1 < 24 and key(m2 + 1) == k:
                m2 += 1
            if best is None or m2 - m > best[1] - best[0]:
                best = (m, m2)
            m = m2 + 1
        m0, m1 = best
        taps = [(c0 + 3 * m0, w) for c0, w in key(m0)]
        odd = [(m, rows[m]) for m in range(24) if not (m0 <= m <= m1)]
        # verify reconstruction
        for m in range(m0, m1 + 1):
            want = dict(rows[m])
            got = {c0 + 3 * (m - m0): w for c0, w in taps}
            assert set(got) == set(want) and all(
                abs(got[c] - want[c]) < 1e-9 for c in got), (r, m)
        plan[r] = dict(m0=m0, cnt=m1 - m0 + 1, taps=taps, odd=odd)
    return plan


SI2PLAN = _si2_plan()
# slot table for si2 weights (slot idx within si2 region, offset 24)
_SI2_SLOTS = {}
for r in range(4):
    for _, w in SI2PLAN[r]["taps"]:
        _SI2_SLOTS.setdefault(round(w, 9), len(_SI2_SLOTS))
    for _, cws in SI2PLAN[r]["odd"]:
        for _, w in cws:
            _SI2_SLOTS.setdefault(round(w, 9), len(_SI2_SLOTS))


# ---------------- program ----------------
@functools.lru_cache(maxsize=1)
def _build():
    nc = bacc.Bacc("TRN2", target_bir_lowering=False, debug=False)
    dec_d = nc.dram_tensor("dec", [BPC, C, H, W], F32, kind="ExternalInput")
    skip_d = nc.dram_tensor("skip", [BPC, C, H, W], F32, kind="ExternalInput")
    aw_d = nc.dram_tensor("aw", [96, NSLOT, 96], BF16, kind="ExternalInput")
    mw_d = nc.dram_tensor("mw", [128, 8, 128], BF16, kind="ExternalInput")
    coef_d = nc.dram_tensor("coef", [128, 8], F32, kind="ExternalInput")
    out_d = nc.dram_tensor("out", [BPC, C, H, W], F32, kind="ExternalOutput")

    with tile.TileContext(nc) as tc:
        with (
            tc.tile_pool(name="const", bufs=1) as pconst,
            tc.tile_pool(name="ystage", bufs=2) as pystage,
            tc.tile_pool(name="yfull", bufs=1) as pyfull,
            tc.tile_pool(name="pyw", bufs=1) as pyw,
            tc.tile_pool(name="phd", bufs=1) as phd,
            tc.tile_pool(name="pslab", bufs=2) as pslab,
            tc.tile_pool(name="pdl", bufs=1) as pdl,
            tc.tile_pool(name="pwcon", bufs=1) as pwcon,
            tc.tile_pool(name="pdog", bufs=2) as pdog,
            tc.tile_pool(name="pmixs", bufs=2) as pmixs,
            tc.tile_pool(name="ps_tr", bufs=1, space="PSUM") as ps_tr,
            tc.tile_pool(name="ps_d", bufs=2, space="PSUM") as ps_d,
            tc.tile_pool(name="ps_wed", bufs=1, space="PSUM") as ps_wed,
            tc.tile_pool(name="ps_mix", bufs=2, space="PSUM") as ps_mix,
        ):
            aw_sb = pconst.tile([96, NSLOT, 96], BF16)
            nc.sync.dma_start(out=aw_sb[:], in_=aw_d[:])
            mw_sb = pconst.tile([128, 8, 128], BF16)
            nc.sync.dma_start(out=mw_sb[:], in_=mw_d[:])
            coef_sb = pconst.tile([128, 8], F32)
            nc.sync.dma_start(out=coef_sb[:], in_=coef_d[:])
            ident = pconst.tile([128, 128], BF16)
            make_identity(nc, ident[:])

            def cf(cb, j):
                return coef_sb[:, cb * 4 + j : cb * 4 + j + 1]

            def emit_y(s, cb):
                """y16 [128,96,96] bf16 via DMA + gpsimd accum-DMA + DVE cast."""
                csl = slice(cb * 128, (cb + 1) * 128)
                y16 = pyfull.tile([128, H, W], BF16, tag=f"y{cb}", name=f"y_{s}_{cb}")
                for st in range(24):
                    rsl = slice(st * 4, st * 4 + 4)
                    d32 = pystage.tile([128, 4, W], F32, tag="d32",
                                       name=f"d32_{s}_{cb}_{st}")
                    nc.sync.dma_start(out=d32[:], in_=dec_d[s, csl, rsl])
                    k32 = pystage.tile([128, 4, W], F32, tag="k32",
                                       name=f"k32_{s}_{cb}_{st}")
                    nc.sync.dma_start(out=k32[:], in_=skip_d[s, csl, rsl])
                    nc.vector.tensor_add(out=y16[:, rsl, :], in0=d32[:], in1=k32[:])
                return y16

            def emit_transpose(s, cb, y16):
                yw = pyw.tile([96, H, 128], BF16, tag="yw", name=f"yw_{s}_{cb}")
                for g in range(12):
                    pt = ps_tr.tile([96, 8, 128], BF16, tag="tr")
                    for k in range(8):
                        nc.tensor.transpose(pt[:, k, :], y16[:, g * 8 + k, :],
                                            ident[:])
                    nc.vector.tensor_copy(out=yw[:, g * 8 : g * 8 + 8, :], in_=pt[:])
                return yw

            def emit_hd(s, cb, yw):
                hds = []
                for si in range(3):
                    hd = phd.tile([96, NS[si], 128], BF16, tag=f"hd{si}",
                                  name=f"hd{si}_{s}_{cb}")
                    if si == 0:
                        nc.vector.tensor_add(out=hd[:], in0=yw[:, _sl(1, 4, 24), :],
                                             in1=yw[:, _sl(2, 4, 24), :])
                    elif si == 1:
                        nc.vector.tensor_add(out=hd[:], in0=yw[:, _sl(0, 2, 48), :],
                                             in1=yw[:, _sl(1, 2, 48), :])
                    else:
                        for q, f in ((0, 1 / 6), (1, 0.5), (2, 5 / 6)):
                            a = yw[:, _sl(q, 4, 24), :]
                            b = yw[:, _sl(q + 1, 4, 24), :]
                            if q == 1:
                                nc.vector.tensor_add(out=hd[:, _sl(1, 3, 24), :],
                                                     in0=a, in1=b)
                            else:
                                for hh in range(2):
                                    qsl = slice(hh * 12, hh * 12 + 12)
                                    dl = pdl.tile([96, 12, 128], BF16, tag="dl75")
                                    nc.vector.tensor_sub(
                                        out=dl[:], in0=b[:, qsl, :], in1=a[:, qsl, :])
                                    nc.vector._custom_dve(
                                        AFFINE_THEN_ADD,
                                        out=hd[:, _sl(q + 36 * hh, 3, 12), :],
                                        in0=dl[:], in1=a[:, qsl, :],
                                        s0=float(f), s1=0.0)
                    hds.append(hd)
                return hds

            def emit_fea(s, cb, yw, hds, wcon):
                cp_by_j = [dict(UP[si][1]) for si in range(3)]
                for r in range(4):
                    for g in range(3):
                        mlo = g * 8
                        es = []
                        for si in range(3):
                            pd = ps_d.tile([96, 8, 128], F32, tag="d")
                            runs, _ = UP[si]
                            for hb in range(2):
                                blo = mlo + hb * 4
                                psl = slice(hb * 4, hb * 4 + 4)
                                nc.tensor.matmul(
                                    pd[:, psl, :], lhsT=ident[0:96, 0:96],
                                    rhs=yw[:, _sl(4 * blo + r, 4, 4), :],
                                    start=True, stop=False)
                                mms = []
                                if r in runs:
                                    m0, cnt, a0, S, f = runs[r]
                                    ilo = max(blo, m0)
                                    ihi = min(blo + 4, m0 + cnt)
                                    if ihi > ilo:
                                        n = ihi - ilo
                                        av = a0 + S * (ilo - m0)
                                        osl = pd[:, ilo - mlo : ihi - mlo, :]
                                        base = si * 10 + 2 * r
                                        mms.append((osl, base,
                                                    hds[si][:, _sl(av, S, n), :]))
                                        mms.append((osl, base + 1,
                                                    hds[si][:, _sl(av + 1, S, n), :]))
                                for m in range(blo, blo + 4):
                                    j = 4 * m + r
                                    if j in cp_by_j[si]:
                                        src = cp_by_j[si][j]
                                        slot = si * 10 + 8
                                        if si == 2 and src % 3 == 1:
                                            slot = si * 10 + 9
                                        mms.append((pd[:, m - mlo, :], slot,
                                                    hds[si][:, src, :]))
                                for i, (osl, slot, rhs) in enumerate(mms):
                                    nc.tensor.matmul(
                                        osl, lhsT=aw_sb[:, slot, :], rhs=rhs,
                                        start=False, stop=(i == len(mms) - 1))
                            e = pslab.tile([96, 8, 128], BF16, tag=f"e{si}")
                            nc.scalar.activation(e[:], pd[:], AF.Abs)
                            es.append(e)
                        mx = pslab.tile([96, 8, 128], BF16, tag="mx")
                        nc.vector.tensor_tensor(out=mx[:], in0=es[0][:],
                                                in1=es[1][:], op=AL.max)
                        nc.vector.tensor_tensor(out=mx[:], in0=mx[:],
                                                in1=es[2][:], op=AL.max)
                        mn = pslab.tile([96, 8, 128], BF16, tag="mn")
                        nc.vector.tensor_tensor(out=mn[:], in0=es[0][:],
                                                in1=es[1][:], op=AL.min)
                        nc.vector.tensor_tensor(out=mn[:], in0=mn[:],
                                                in1=es[2][:], op=AL.min)
                        wg = pslab.tile([96, 8, 128], BF16, tag="mx")
                        nc.vector.tensor_sub(out=wg[:], in0=mx[:], in1=mn[:])
                        pw = ps_wed.tile([128, 8, 96], BF16, tag="wed")
                        for k in range(8):
                            nc.tensor.transpose(pw[:, k, :], wg[:, k, :],
                                                ident[0:96, 0:96])
                        nc.vector.tensor_copy(
                            out=wcon[:, _sl(4 * mlo + r, 4, 8), :], in_=pw[:])

            def emit_dog_mix(s, y16s, wcons):
                out_f = [out_d[s, slice(ob * 128, (ob + 1) * 128)]
                         .rearrange("c h w -> c (h w)") for ob in range(2)]
                skip_f = [skip_d[s, slice(ob * 128, (ob + 1) * 128)]
                          .rearrange("c h w -> c (h w)") for ob in range(2)]
                for st in range(12):
                    h0 = st * 8
                    hsl = slice(h0, h0 + 8)
                    r0, r1 = max(h0 - 1, 0), min(h0 + 9, H)
                    zsts = []
                    for cb in range(2):
                        y16, wcon = y16s[cb], wcons[cb]
                        # v with halo rows [r0, r1): vh[k] = S_h y at row r0+k
                        nv = r1 - r0
                        vh = pdog.tile([128, 10, W], BF16, tag="vh")
                        g0, g1 = max(r0, 1), min(r1, 95)
                        nc.vector.tensor_add(
                            out=vh[:, g0 - r0 : g1 - r0, :],
                            in0=y16[:, g0 - 1 : g1 - 1, :],
                            in1=y16[:, g0 + 1 : g1 + 1, :])
                        if r0 == 0:
                            nc.vector.tensor_copy(out=vh[:, 0, :], in_=y16[:, 1, :])
                        if r1 == H:
                            nc.vector.tensor_copy(out=vh[:, nv - 1, :],
                                                  in_=y16[:, 94, :])
                        ctr = slice(h0 - r0, h0 - r0 + 8)
                        # u stripe (8 rows): S_w y
                        ust = pdog.tile([128, 8, W], BF16, tag="ust")
                        nc.gpsimd.tensor_tensor(out=ust[:, :, 1:95],
                                                in0=y16[:, hsl, 0:94],
                                                in1=y16[:, hsl, 2:96], op=AL.add)
                        nc.gpsimd.tensor_copy(out=ust[:, :, 0],
                                              in_=y16[:, hsl, 1])
                        nc.gpsimd.tensor_copy(out=ust[:, :, 95],
                                              in_=y16[:, hsl, 94])
                        # c1 = u + v (gpsimd)
                        c1 = pdog.tile([128, 8, W], BF16, tag="c1")
                        nc.gpsimd.tensor_tensor(out=c1[:], in0=ust[:],
                                                in1=vh[:, ctr, :], op=AL.add)
                        # Q = S_w v (center rows)
                        qt = pdog.tile([128, 8, W], BF16, tag="qt")
                        nc.vector.tensor_add(out=qt[:, :, 1:95],
                                             in0=vh[:, ctr, 0:94],
                                             in1=vh[:, ctr, 2:96])
                        nc.vector.tensor_copy(out=qt[:, :, 0], in_=vh[:, ctr, 1])
                        nc.vector.tensor_copy(out=qt[:, :, 95],
                                              in_=vh[:, ctr, 94])
                        t2 = pdog.tile([128, 8, W], BF16, tag="t2")
                        nc.scalar.activation(t2[:], c1[:], AF.Copy,
                                             scale=cf(cb, 1))
                        z1 = pdog.tile([128, 8, W], BF16, tag="qt")
                        nc.vector._custom_dve(AFFINE_THEN_ADD, out=z1[:],
                                              in0=qt[:], in1=t2[:],
                                              s0=cf(cb, 2), s1=0.0)
                        zst = pdog.tile([128, 8, W], BF16, tag=f"zst{cb}")
                        nc.vector._custom_dve(AFFINE_THEN_ADD, out=zst[:],
                                              in0=y16[:, hsl, :], in1=z1[:],
                                              s0=cf(cb, 3), s1=0.0)
                        zsts.append(zst)
                    z0f = zsts[0][:].rearrange("c h w -> c (h w)")
                    z1f = zsts[1][:].rearrange("c h w -> c (h w)")
                    w0f = wcons[0][:].rearrange("c h w -> c (h w)")
                    w1f = wcons[1][:].rearrange("c h w -> c (h w)")
                    for ob in range(2):
                        col0 = h0 * W
                        csl = slice(col0, col0 + 768)
                        sk32 = pmixs.tile([128, 768], F32, tag="sk32")
                        nc.sync.dma_start(out=sk32[:], in_=skip_f[ob][:, csl])
                        sk = pmixs.tile([128, 768], BF16, tag="sk")
                        nc.vector.tensor_copy(out=sk[:], in_=sk32[:])
                        ot = pmixs.tile([128, 768], F32, tag="ot")
                        for hf in range(2):
                            lsl = slice(hf * 384, hf * 384 + 384)
                            pmx = ps_mix.tile([128, 384], F32, tag="pmix")
                            nc.tensor.matmul(pmx[:], lhsT=ident[:],
                                             rhs=sk[:, lsl],
                                             start=True, stop=False)
                            nc.tensor.matmul(pmx[:], lhsT=mw_sb[:, ob, :],
                                             rhs=z0f[:, lsl],
                                             start=False, stop=False)
                            nc.tensor.matmul(pmx[:], lhsT=mw_sb[:, 2 + ob, :],
                                             rhs=z1f[:, lsl],
                                             start=False, stop=False)
                            nc.tensor.matmul(pmx[:], lhsT=mw_sb[:, 4 + ob, :],
                                             rhs=w0f[:, csl][:, lsl],
                                             start=False, stop=False)
                            nc.tensor.matmul(pmx[:], lhsT=mw_sb[:, 6 + ob, :],
                                             rhs=w1f[:, csl][:, lsl],
                                             start=False, stop=True)
                            nc.scalar.copy(ot[:, lsl], pmx[:])
                        nc.scalar.dma_start(out=out_f[ob][:, csl], in_=ot[:])

            for s in range(BPC):
                y16s, wcons = {}, {}
                for cb in range(2):
                    y16 = emit_y(s, cb)
                    y16s[cb] = y16
                    yw = emit_transpose(s, cb, y16)
                    hds = emit_hd(s, cb, yw)
                    wcon = pwcon.tile([128, H, W], BF16, tag=f"wc{cb}",
                                      name=f"wc_{s}_{cb}")
                    wcons[cb] = wcon
                    emit_fea(s, cb, yw, hds, wcon)
                emit_dog_mix(s, y16s, wcons)
    nc.finalize()
    return nc


# ---------------- host consts ----------------
def _consts(w_fea, sigma1, sigma2, mixer_w):
    wf = w_fea.reshape(C).astype(np.float64) * (2.0 / 3.0)
    t, n = {}, {}
    for i, sg in ((1, sigma1), (2, sigma2)):
        sig = 2.0 / (1.0 + np.exp(-sg.reshape(C).astype(np.float64)))
        t[i] = np.exp(-1.0 / (2.0 * sig ** 2))
        n[i] = (1.0 + 2.0 * t[i]) ** -2
    alpha = n[1] - n[2]
    beta = n[1] * t[1] - n[2] * t[2]
    gamma = n[1] * t[1] ** 2 - n[2] * t[2] ** 2

    coef = np.zeros((128, 8), dtype=np.float32)
    for cb in range(2):
        ch = slice(cb * 128, (cb + 1) * 128)
        coef[:, cb * 4 + 0] = wf[ch]
        coef[:, cb * 4 + 1] = beta[ch]
        coef[:, cb * 4 + 2] = gamma[ch]
        coef[:, cb * 4 + 3] = 3.0 + alpha[ch]

    aw = np.zeros((96, NSLOT, 96), dtype=np.float64)
    for si in range(3):
        ns = NS[si]
        A = _resize_matrix(ns, H) @ _resize_matrix(H, ns)
        fold = 0.5 if si in (0, 1) else 1.0
        Af = (fold * A).T
        runs, _ = _up_runs(ns)
        base = si * 10
        for r, (m0, cnt, a0, S, f) in runs.items():
            if si == 2:
                wa = (1.0 - f) * _si2_scale(a0)
                wb = f * _si2_scale(a0 + 1)
            else:
                wa, wb = (1.0 - f), f
            aw[:, base + 2 * r, :] = -wa * Af
            aw[:, base + 2 * r + 1, :] = -wb * Af
        aw[:, base + 8, :] = -Af
        aw[:, base + 9, :] = -0.5 * Af
    aw = aw.astype(ml_dtypes.bfloat16)

    M = mixer_w.reshape(C, C).astype(np.float64)
    mw = np.zeros((128, 8, 128), dtype=np.float64)
    for kc in range(2):
        for ob in range(2):
            blk = M[ob * 128:(ob + 1) * 128, kc * 128:(kc + 1) * 128].T
            mw[:, kc * 2 + ob, :] = blk
            mw[:, 4 + kc * 2 + ob, :] = wf[kc * 128:(kc + 1) * 128, None] * blk
    mw = mw.astype(ml_dtypes.bfloat16)
    return aw, mw, coef


def kernel(skip, dec, w_fea, sigma1, sigma2, mixer_w, _trace=[False]):
    skip = np.ascontiguousarray(np.asarray(skip, dtype=np.float32))
    dec = np.ascontiguousarray(np.asarray(dec, dtype=np.float32))
    aw, mw, coef = _consts(
        np.asarray(w_fea), np.asarray(sigma1), np.asarray(sigma2),
        np.asarray(mixer_w))
    nc = _build()
    in_maps = []
    for i in range(NCORES):
        in_maps.append({
            "dec": dec[BPC * i : BPC * (i + 1)],
            "skip": skip[BPC * i : BPC * (i + 1)],
            "aw": aw, "mw": mw, "coef": coef,
        })
    res = run_bass_kernel_spmd(nc, in_maps, core_ids=list(range(NCORES)),
                               trace=_trace[0])
    kernel.last_result = res
    return np.concatenate([r["out"] for r in res.results], axis=0)


kernel.last_result = None


# revision 4
# speedup vs baseline: 1.3252x; 1.0052x over previous
"""Trainium2 Bass kernel v2 for nn_DSEBlock — engine-balanced redesign.

Decomposition (validated in mirror.py vs reference):
  y = dec + skip                         (DMA-accumulated, fp32, cast bf16)
  FEA: per scale si: psum = y - U_si(hd_si) x A_w,si  (PE, planar 4-row banks)
       e_si = |psum| (ACT abs) ; wedge = max3(e) - min3(e) (DVE/GP slab TT)
       wcon[c,h,w] = XBAR-DMA row transposes of wedge slabs
  DoG: u = S_w y (DVE), v = S_h y (GP), c1 = v + u (gpsimd accum-DMA),
       Q = S_h u (DVE)
  z   = (3+a)y + b*c1 + g*Q + wf'*wcon   (AFFINE_THEN_ADD chain + ACT)
  out = M z + skip                       (PE mixer, DVE psum+skip extract)

Engines: PE ~250us, DVE ~300, ACT ~280, GP ~240, DMA ~270 per core (est).
"""
import functools

import ml_dtypes
import numpy as np

import concourse.bass as bass
import concourse.mybir as mybir
import concourse.tile as tile
from concourse import bacc
from concourse.bass import ts
from concourse.bass_utils import run_bass_kernel_spmd
from concourse.dve_ops import AFFINE_THEN_ADD
from concourse.masks import make_identity

F32 = mybir.dt.float32
BF16 = mybir.dt.bfloat16
AL = mybir.AluOpType
AF = mybir.ActivationFunctionType

B, C, H, W = 16, 256, 96, 96
NCORES = 8
BPC = B // NCORES
SCALES = [0.25, 0.5, 0.75]
NS = [24, 48, 72]
HW = H * W


def _sl(start, step, cnt):
    return slice(start, start + step * (cnt - 1) + 1, step)


# ---------------- host-side resize structure ----------------
def _resize_matrix(n_in, n_out):
    A = np.zeros((n_out, n_in), dtype=np.float64)
    scale = n_in / n_out
    for j in range(n_out):
        x = (j + 0.5) * scale - 0.5
        i0 = int(np.floor(x))
        f = x - i0
        A[j, min(max(i0, 0), n_in - 1)] += 1.0 - f
        A[j, min(max(i0 + 1, 0), n_in - 1)] += f
    return A


def _up_runs(ns):
    scale = ns / H
    S = {24: 1, 48: 2, 72: 3}[ns]
    groups, copies = {}, []
    for j in range(H):
        x = (j + 0.5) * scale - 0.5
        i0 = int(np.floor(x))
        f = x - i0
        if i0 < 0:
            copies.append((j, 0))
            continue
        if i0 + 1 > ns - 1:
            copies.append((j, ns - 1))
            continue
        groups.setdefault(j % 4, []).append((j // 4, i0, f))
    runs = {}
    for r, items in groups.items():
        items.sort()
        ms = [m for m, _, _ in items]
        assert ms == list(range(ms[0], ms[-1] + 1))
        runs[r] = (ms[0], len(ms), items[0][1], S, items[0][2])
    return runs, copies


# aw slot map: per si: base = si*12: slots base+2r (tap a), base+2r+1 (tap b),
# base+8: clamp full, base+9: clamp half (si2 residue-1 sources)
NSLOT = 30


def _ywrows(yw, start, step, cnt):
    return yw[:, _sl(start, step, cnt), :]


def _si2_scale(a):
    return 0.5 if a % 3 == 1 else 1.0


UP = [_up_runs(ns) for ns in NS]


def _si2_plan():
    """B75 = U75@D75 folded tap plan: per r: interior run (m0, cnt,
    taps=[(c0, w)..]) with cols c0+3m, plus odd rows [(m, [(col, w)..])]."""
    B = _resize_matrix(72, 96) @ _resize_matrix(96, 72)
    plan = {}
    for r in range(4):
        rows = {}
        for m in range(24):
            j = 4 * m + r
            cols = np.nonzero(np.abs(B[j]) > 1e-12)[0]
            rows[m] = [(int(c), float(B[j, c])) for c in cols]
        # find the longest run of m where (cols - 3m, weights) identical
        def key(m):
            return tuple((c - 3 * m, round(w, 9)) for c, w in rows[m])
        best = None
        m = 0
        while m < 24:
            k = key(m)
            m2 = m
            while m2 + 1 < 24 and key(m2 + 1) == k:
                m2 += 1
            if best is None or m2 - m > best[1] - best[0]:
                best = (m, m2)
            m = m2 + 1
        m0, m1 = best
        taps = [(c0 + 3 * m0, w) for c0, w in key(m0)]
        odd = [(m, rows[m]) for m in range(24) if not (m0 <= m <= m1)]
        # verify reconstruction
        for m in range(m0, m1 + 1):
            want = dict(rows[m])
            got = {c0 + 3 * (m - m0): w for c0, w in taps}
            assert set(got) == set(want) and all(
                abs(got[c] - want[c]) < 1e-9 for c in got), (r, m)
        plan[r] = dict(m0=m0, cnt=m1 - m0 + 1, taps=taps, odd=odd)
    return plan


SI2PLAN = _si2_plan()
# slot table for si2 weights (slot idx within si2 region, offset 24)
_SI2_SLOTS = {}
for r in range(4):
    for _, w in SI2PLAN[r]["taps"]:
        _SI2_SLOTS.setdefault(round(w, 9), len(_SI2_SLOTS))
    for _, cws in SI2PLAN[r]["odd"]:
        for _, w in cws:
            _SI2_SLOTS.setdefault(round(w, 9), len(_SI2_SLOTS))


# ---------------- program ----------------
@functools.lru_cache(maxsize=1)
def _build():
    nc = bacc.Bacc("TRN2", target_bir_lowering=False, debug=False)
    dec_d = nc.dram_tensor("dec", [BPC, C, H, W], F32, kind="ExternalInput")
    skip_d = nc.dram_tensor("skip", [BPC, C, H, W], F32, kind="ExternalInput")
    aw_d = nc.dram_tensor("aw", [96, NSLOT, 96], BF16, kind="ExternalInput")
    mw_d = nc.dram_tensor("mw", [128, 12, 128], BF16, kind="ExternalInput")
    coef_d = nc.dram_tensor("coef", [128, 8], F32, kind="ExternalInput")
    out_d = nc.dram_tensor("out", [BPC, C, H, W], F32, kind="ExternalOutput")

    with tile.TileContext(nc) as tc:
        with (
            tc.tile_pool(name="const", bufs=1) as pconst,
            tc.tile_pool(name="ystage", bufs=2) as pystage,
            tc.tile_pool(name="yfull", bufs=1) as pyfull,
            tc.tile_pool(name="pyw", bufs=1) as pyw,
            tc.tile_pool(name="phd", bufs=1) as phd,
            tc.tile_pool(name="pslab", bufs=2) as pslab,
            tc.tile_pool(name="pdl", bufs=1) as pdl,
            tc.tile_pool(name="pwcon", bufs=1) as pwcon,
            tc.tile_pool(name="pdog", bufs=2) as pdog,
            tc.tile_pool(name="pmixs", bufs=2) as pmixs,
            tc.tile_pool(name="ps_tr", bufs=1, space="PSUM") as ps_tr,
            tc.tile_pool(name="ps_d", bufs=2, space="PSUM") as ps_d,
            tc.tile_pool(name="ps_wed", bufs=1, space="PSUM") as ps_wed,
            tc.tile_pool(name="ps_mix", bufs=2, space="PSUM") as ps_mix,
        ):
            aw_sb = pconst.tile([96, NSLOT, 96], BF16)
            nc.sync.dma_start(out=aw_sb[:], in_=aw_d[:])
            mw_sb = pconst.tile([128, 12, 128], BF16)
            nc.sync.dma_start(out=mw_sb[:], in_=mw_d[:])
            coef_sb = pconst.tile([128, 8], F32)
            nc.sync.dma_start(out=coef_sb[:], in_=coef_d[:])
            ident = pconst.tile([128, 128], BF16)
            make_identity(nc, ident[:])

            def cf(cb, j):
                return coef_sb[:, cb * 4 + j : cb * 4 + j + 1]

            def emit_y(s, cb):
                """y16 [128,96,96] bf16 via DMA + gpsimd accum-DMA + DVE cast."""
                csl = slice(cb * 128, (cb + 1) * 128)
                y16 = pyfull.tile([128, H, W], BF16, tag=f"y{cb}", name=f"y_{s}_{cb}")
                for st in range(24):
                    rsl = slice(st * 4, st * 4 + 4)
                    d32 = pystage.tile([128, 4, W], F32, tag="d32",
                                       name=f"d32_{s}_{cb}_{st}")
                    nc.sync.dma_start(out=d32[:], in_=dec_d[s, csl, rsl])
                    k32 = pystage.tile([128, 4, W], F32, tag="k32",
                                       name=f"k32_{s}_{cb}_{st}")
                    nc.sync.dma_start(out=k32[:], in_=skip_d[s, csl, rsl])
                    nc.vector.tensor_add(out=y16[:, rsl, :], in0=d32[:], in1=k32[:])
                return y16

            def emit_transpose(s, cb, y16):
                yw = pyw.tile([96, H, 128], BF16, tag="yw", name=f"yw_{s}_{cb}")
                for g in range(12):
                    pt = ps_tr.tile([96, 8, 128], BF16, tag="tr")
                    for k in range(8):
                        nc.tensor.transpose(pt[:, k, :], y16[:, g * 8 + k, :],
                                            ident[:])
                    nc.vector.tensor_copy(out=yw[:, g * 8 : g * 8 + 8, :], in_=pt[:])
                return yw

            def emit_hd(s, cb, yw):
                hds = []
                for si in range(3):
                    hd = phd.tile([96, NS[si], 128], BF16, tag=f"hd{si}",
                                  name=f"hd{si}_{s}_{cb}")
                    if si == 0:
                        nc.vector.tensor_add(out=hd[:], in0=yw[:, _sl(1, 4, 24), :],
                                             in1=yw[:, _sl(2, 4, 24), :])
                    elif si == 1:
                        nc.vector.tensor_add(out=hd[:], in0=yw[:, _sl(0, 2, 48), :],
                                             in1=yw[:, _sl(1, 2, 48), :])
                    else:
                        for q, f in ((0, 1 / 6), (1, 0.5), (2, 5 / 6)):
                            a = yw[:, _sl(q, 4, 24), :]
                            b = yw[:, _sl(q + 1, 4, 24), :]
                            if q == 1:
                                nc.vector.tensor_add(out=hd[:, _sl(1, 3, 24), :],
                                                     in0=a, in1=b)
                            else:
                                for hh in range(2):
                                    qsl = slice(hh * 12, hh * 12 + 12)
                                    dl = pdl.tile([96, 12, 128], BF16, tag="dl75")
                                    nc.vector.tensor_sub(
                                        out=dl[:], in0=b[:, qsl, :], in1=a[:, qsl, :])
                                    nc.vector._custom_dve(
                                        AFFINE_THEN_ADD,
                                        out=hd[:, _sl(q + 36 * hh, 3, 12), :],
                                        in0=dl[:], in1=a[:, qsl, :],
                                        s0=float(f), s1=0.0)
                    hds.append(hd)
                return hds

            def emit_fea(s, cb, yw, hds, wcon):
                cp_by_j = [dict(UP[si][1]) for si in range(3)]
                for r in range(4):
                    for g in range(3):
                        mlo = g * 8
                        es = []
                        for si in range(3):
                            pd = ps_d.tile([96, 8, 128], F32, tag="d")
                            runs, _ = UP[si]
                            for hb in range(2):
                                blo = mlo + hb * 4
                                psl = slice(hb * 4, hb * 4 + 4)
                                nc.tensor.matmul(
                                    pd[:, psl, :], lhsT=ident[0:96, 0:96],
                                    rhs=yw[:, _sl(4 * blo + r, 4, 4), :],
                                    start=True, stop=False)
                                mms = []
                                if r in runs:
                                    m0, cnt, a0, S, f = runs[r]
                                    ilo = max(blo, m0)
                                    ihi = min(blo + 4, m0 + cnt)
                                    if ihi > ilo:
                                        n = ihi - ilo
                                        av = a0 + S * (ilo - m0)
                                        osl = pd[:, ilo - mlo : ihi - mlo, :]
                                        base = si * 10 + 2 * r
                                        mms.append((osl, base,
                                                    hds[si][:, _sl(av, S, n), :]))
                                        mms.append((osl, base + 1,
                                                    hds[si][:, _sl(av + 1, S, n), :]))
                                for m in range(blo, blo + 4):
                                    j = 4 * m + r
                                    if j in cp_by_j[si]:
                                        src = cp_by_j[si][j]
                                        slot = si * 10 + 8
                                        if si == 2 and src % 3 == 1:
                                            slot = si * 10 + 9
                                        mms.append((pd[:, m - mlo, :], slot,
                                                    hds[si][:, src, :]))
                                for i, (osl, slot, rhs) in enumerate(mms):
                                    nc.tensor.matmul(
                                        osl, lhsT=aw_sb[:, slot, :], rhs=rhs,
                                        start=False, stop=(i == len(mms) - 1))
                            e = pslab.tile([96, 8, 128], BF16, tag=f"e{si}")
                            nc.scalar.activation(e[:], pd[:], AF.Abs)
                            es.append(e)
                        mx = pslab.tile([96, 8, 128], BF16, tag="mx")
                        nc.vector.tensor_tensor(out=mx[:], in0=es[0][:],
                                                in1=es[1][:], op=AL.max)
                        nc.vector.tensor_tensor(out=mx[:], in0=mx[:],
                                                in1=es[2][:], op=AL.max)
                        mn = pslab.tile([96, 8, 128], BF16, tag="mn")
                        nc.vector.tensor_tensor(out=mn[:], in0=es[0][:],
                                                in1=es[1][:], op=AL.min)
                        nc.vector.tensor_tensor(out=mn[:], in0=mn[:],
                                                in1=es[2][:], op=AL.min)
                        wg = pslab.tile([96, 8, 128], BF16, tag="mx")
                        nc.vector.tensor_sub(out=wg[:], in0=mx[:], in1=mn[:])
                        pw = ps_wed.tile([128, 8, 96], BF16, tag="wed")
                        for k in range(8):
                            nc.tensor.transpose(pw[:, k, :], wg[:, k, :],
                                                ident[0:96, 0:96])
                        nc.vector.tensor_copy(
                            out=wcon[:, _sl(4 * mlo + r, 4, 8), :], in_=pw[:])

            def emit_dog_mix(s, y16s, wcons):
                out_f = [out_d[s, slice(ob * 128, (ob + 1) * 128)]
                         .rearrange("c h w -> c (h w)") for ob in range(2)]
                skip_f = [skip_d[s, slice(ob * 128, (ob + 1) * 128)]
                          .rearrange("c h w -> c (h w)") for ob in range(2)]
                for st in range(12):
                    h0 = st * 8
                    hsl = slice(h0, h0 + 8)
                    r0, r1 = max(h0 - 1, 0), min(h0 + 9, H)
                    zsts = []
                    for cb in range(2):
                        y16, wcon = y16s[cb], wcons[cb]
                        # v with halo rows [r0, r1): vh[k] = S_h y at row r0+k
                        nv = r1 - r0
                        vh = pdog.tile([128, 10, W], BF16, tag="vh")
                        g0, g1 = max(r0, 1), min(r1, 95)
                        nc.vector.tensor_add(
                            out=vh[:, g0 - r0 : g1 - r0, :],
                            in0=y16[:, g0 - 1 : g1 - 1, :],
                            in1=y16[:, g0 + 1 : g1 + 1, :])
                        if r0 == 0:
                            nc.vector.tensor_copy(out=vh[:, 0, :], in_=y16[:, 1, :])
                        if r1 == H:
                            nc.vector.tensor_copy(out=vh[:, nv - 1, :],
                                                  in_=y16[:, 94, :])
                        ctr = slice(h0 - r0, h0 - r0 + 8)
                        # u stripe (8 rows): S_w y
                        ust = pdog.tile([128, 8, W], BF16, tag="ust")
                        nc.gpsimd.tensor_tensor(out=ust[:, :, 1:95],
                                                in0=y16[:, hsl, 0:94],
                                                in1=y16[:, hsl, 2:96], op=AL.add)
                        nc.gpsimd.tensor_copy(out=ust[:, :, 0],
                                              in_=y16[:, hsl, 1])
                        nc.gpsimd.tensor_copy(out=ust[:, :, 95],
                                              in_=y16[:, hsl, 94])
                        # c1 = u + v (gpsimd)
                        c1 = pdog.tile([128, 8, W], BF16, tag="c1")
                        nc.gpsimd.tensor_tensor(out=c1[:], in0=ust[:],
                                                in1=vh[:, ctr, :], op=AL.add)
                        # Q = S_w v (center rows)
                        qt = pdog.tile([128, 8, W], BF16, tag="qt")
                        nc.vector.tensor_add(out=qt[:, :, 1:95],
                                             in0=vh[:, ctr, 0:94],
                                             in1=vh[:, ctr, 2:96])
                        nc.vector.tensor_copy(out=qt[:, :, 0], in_=vh[:, ctr, 1])
                        nc.vector.tensor_copy(out=qt[:, :, 95],
                                              in_=vh[:, ctr, 94])
                        t2 = pdog.tile([128, 8, W], BF16, tag="t2")
                        nc.scalar.activation(t2[:], c1[:], AF.Copy,
                                             scale=cf(cb, 1))
                        zst = pdog.tile([128, 8, W], BF16, tag=f"zst{cb}")
                        nc.vector._custom_dve(AFFINE_THEN_ADD, out=zst[:],
                                              in0=qt[:], in1=t2[:],
                                              s0=cf(cb, 2), s1=0.0)
                        zsts.append(zst)
                    z0f = zsts[0][:].rearrange("c h w -> c (h w)")
                    z1f = zsts[1][:].rearrange("c h w -> c (h w)")
                    w0f = wcons[0][:].rearrange("c h w -> c (h w)")
                    w1f = wcons[1][:].rearrange("c h w -> c (h w)")
                    y0f = y16s[0][:].rearrange("c h w -> c (h w)")
                    y1f = y16s[1][:].rearrange("c h w -> c (h w)")
                    for ob in range(2):
                        col0 = h0 * W
                        csl = slice(col0, col0 + 768)
                        sk32 = pmixs.tile([128, 768], F32, tag="sk32")
                        nc.sync.dma_start(out=sk32[:], in_=skip_f[ob][:, csl])
                        sk = pmixs.tile([128, 768], BF16, tag="sk")
                        nc.vector.tensor_copy(out=sk[:], in_=sk32[:])
                        ot = pmixs.tile([128, 768], F32, tag="ot")
                        for hf in range(2):
                            lsl = slice(hf * 384, hf * 384 + 384)
                            pmx = ps_mix.tile([128, 384], F32, tag="pmix")
                            nc.tensor.matmul(pmx[:], lhsT=ident[:],
                                             rhs=sk[:, lsl],
                                             start=True, stop=False)
                            nc.tensor.matmul(pmx[:], lhsT=mw_sb[:, ob, :],
                                             rhs=z0f[:, lsl],
                                             start=False, stop=False)
                            nc.tensor.matmul(pmx[:], lhsT=mw_sb[:, 2 + ob, :],
                                             rhs=z1f[:, lsl],
                                             start=False, stop=False)
                            nc.tensor.matmul(pmx[:], lhsT=mw_sb[:, 4 + ob, :],
                                             rhs=w0f[:, csl][:, lsl],
                                             start=False, stop=False)
                            nc.tensor.matmul(pmx[:], lhsT=mw_sb[:, 6 + ob, :],
                                             rhs=w1f[:, csl][:, lsl],
                                             start=False, stop=False)
                            nc.tensor.matmul(pmx[:], lhsT=mw_sb[:, 8 + ob, :],
                                             rhs=y0f[:, csl][:, lsl],
                                             start=False, stop=False)
                            nc.tensor.matmul(pmx[:], lhsT=mw_sb[:, 10 + ob, :],
                                             rhs=y1f[:, csl][:, lsl],
                                             start=False, stop=True)
                            nc.scalar.copy(ot[:, lsl], pmx[:])
                        nc.scalar.dma_start(out=out_f[ob][:, csl], in_=ot[:])

            for s in range(BPC):
                y16s, wcons = {}, {}
                for cb in range(2):
                    y16 = emit_y(s, cb)
                    y16s[cb] = y16
                    yw = emit_transpose(s, cb, y16)
                    hds = emit_hd(s, cb, yw)
                    wcon = pwcon.tile([128, H, W], BF16, tag=f"wc{cb}",
                                      name=f"wc_{s}_{cb}")
                    wcons[cb] = wcon
                    emit_fea(s, cb, yw, hds, wcon)
                emit_dog_mix(s, y16s, wcons)
    nc.finalize()
    return nc


# ---------------- host consts ----------------
def _consts(w_fea, sigma1, sigma2, mixer_w):
    wf = w_fea.reshape(C).astype(np.float64) * (2.0 / 3.0)
    t, n = {}, {}
    for i, sg in ((1, sigma1), (2, sigma2)):
        sig = 2.0 / (1.0 + np.exp(-sg.reshape(C).astype(np.float64)))
        t[i] = np.exp(-1.0 / (2.0 * sig ** 2))
        n[i] = (1.0 + 2.0 * t[i]) ** -2
    alpha = n[1] - n[2]
    beta = n[1] * t[1] - n[2] * t[2]
    gamma = n[1] * t[1] ** 2 - n[2] * t[2] ** 2

    coef = np.zeros((128, 8), dtype=np.float32)
    for cb in range(2):
        ch = slice(cb * 128, (cb + 1) * 128)
        coef[:, cb * 4 + 0] = wf[ch]
        coef[:, cb * 4 + 1] = beta[ch]
        coef[:, cb * 4 + 2] = gamma[ch]
        coef[:, cb * 4 + 3] = 3.0 + alpha[ch]

    aw = np.zeros((96, NSLOT, 96), dtype=np.float64)
    for si in range(3):
        ns = NS[si]
        A = _resize_matrix(ns, H) @ _resize_matrix(H, ns)
        fold = 0.5 if si in (0, 1) else 1.0
        Af = (fold * A).T
        runs, _ = _up_runs(ns)
        base = si * 10
        for r, (m0, cnt, a0, S, f) in runs.items():
            if si == 2:
                wa = (1.0 - f) * _si2_scale(a0)
                wb = f * _si2_scale(a0 + 1)
            else:
                wa, wb = (1.0 - f), f
            aw[:, base + 2 * r, :] = -wa * Af
            aw[:, base + 2 * r + 1, :] = -wb * Af
        aw[:, base + 8, :] = -Af
        aw[:, base + 9, :] = -0.5 * Af
    aw = aw.astype(ml_dtypes.bfloat16)

    M = mixer_w.reshape(C, C).astype(np.float64)
    mw = np.zeros((128, 12, 128), dtype=np.float64)
    a3 = 3.0 + alpha
    for kc in range(2):
        for ob in range(2):
            blk = M[ob * 128:(ob + 1) * 128, kc * 128:(kc + 1) * 128].T
            mw[:, kc * 2 + ob, :] = blk
            mw[:, 4 + kc * 2 + ob, :] = wf[kc * 128:(kc + 1) * 128, None] * blk
            mw[:, 8 + kc * 2 + ob, :] = a3[kc * 128:(kc + 1) * 128, None] * blk
    mw = mw.astype(ml_dtypes.bfloat16)
    return aw, mw, coef


def kernel(skip, dec, w_fea, sigma1, sigma2, mixer_w, _trace=[False]):
    skip = np.ascontiguousarray(np.asarray(skip, dtype=np.float32))
    dec = np.ascontiguousarray(np.asarray(dec, dtype=np.float32))
    aw, mw, coef = _consts(
        np.asarray(w_fea), np.asarray(sigma1), np.asarray(sigma2),
        np.asarray(mixer_w))
    nc = _build()
    in_maps = []
    for i in range(NCORES):
        in_maps.append({
            "dec": dec[BPC * i : BPC * (i + 1)],
            "skip": skip[BPC * i : BPC * (i + 1)],
            "aw": aw, "mw": mw, "coef": coef,
        })
    res = run_bass_kernel_spmd(nc, in_maps, core_ids=list(range(NCORES)),
                               trace=_trace[0])
    kernel.last_result = res
    return np.concatenate([r["out"] for r in res.results], axis=0)


kernel.last_result = None


# revision 5
# speedup vs baseline: 1.3336x; 1.0063x over previous
"""Trainium2 Bass kernel v2 for nn_DSEBlock — engine-balanced redesign.

Decomposition (validated in mirror.py vs reference):
  y = dec + skip                         (DMA-accumulated, fp32, cast bf16)
  FEA: per scale si: psum = y - U_si(hd_si) x A_w,si  (PE, planar 4-row banks)
       e_si = |psum| (ACT abs) ; wedge = max3(e) - min3(e) (DVE/GP slab TT)
       wcon[c,h,w] = XBAR-DMA row transposes of wedge slabs
  DoG: u = S_w y (DVE), v = S_h y (GP), c1 = v + u (gpsimd accum-DMA),
       Q = S_h u (DVE)
  z   = (3+a)y + b*c1 + g*Q + wf'*wcon   (AFFINE_THEN_ADD chain + ACT)
  out = M z + skip                       (PE mixer, DVE psum+skip extract)

Engines: PE ~250us, DVE ~300, ACT ~280, GP ~240, DMA ~270 per core (est).
"""
import functools

import ml_dtypes
import numpy as np

import concourse.bass as bass
import concourse.mybir as mybir
import concourse.tile as tile
from concourse import bacc
from concourse.bass import ts
from concourse.bass_utils import run_bass_kernel_spmd
from concourse.dve_ops import AFFINE_THEN_ADD
from concourse.masks import make_identity

F32 = mybir.dt.float32
BF16 = mybir.dt.bfloat16
AL = mybir.AluOpType
AF = mybir.ActivationFunctionType

B, C, H, W = 16, 256, 96, 96
NCORES = 8
BPC = B // NCORES
SCALES = [0.25, 0.5, 0.75]
NS = [24, 48, 72]
HW = H * W


def _sl(start, step, cnt):
    return slice(start, start + step * (cnt - 1) + 1, step)


# ---------------- host-side resize structure ----------------
def _resize_matrix(n_in, n_out):
    A = np.zeros((n_out, n_in), dtype=np.float64)
    scale = n_in / n_out
    for j in range(n_out):
        x = (j + 0.5) * scale - 0.5
        i0 = int(np.floor(x))
        f = x - i0
        A[j, min(max(i0, 0), n_in - 1)] += 1.0 - f
        A[j, min(max(i0 + 1, 0), n_in - 1)] += f
    return A


def _up_runs(ns):
    scale = ns / H
    S = {24: 1, 48: 2, 72: 3}[ns]
    groups, copies = {}, []
    for j in range(H):
        x = (j + 0.5) * scale - 0.5
        i0 = int(np.floor(x))
        f = x - i0
        if i0 < 0:
            copies.append((j, 0))
            continue
        if i0 + 1 > ns - 1:
            copies.append((j, ns - 1))
            continue
        groups.setdefault(j % 4, []).append((j // 4, i0, f))
    runs = {}
    for r, items in groups.items():
        items.sort()
        ms = [m for m, _, _ in items]
        assert ms == list(range(ms[0], ms[-1] + 1))
        runs[r] = (ms[0], len(ms), items[0][1], S, items[0][2])
    return runs, copies


# aw slot map: per si: base = si*12: slots base+2r (tap a), base+2r+1 (tap b),
# base+8: clamp full, base+9: clamp half (si2 residue-1 sources)
NSLOT = 30


def _ywrows(yw, start, step, cnt):
    return yw[:, _sl(start, step, cnt), :]


def _si2_scale(a):
    return 0.5 if a % 3 == 1 else 1.0


UP = [_up_runs(ns) for ns in NS]


def _si2_plan():
    """B75 = U75@D75 folded tap plan: per r: interior run (m0, cnt,
    taps=[(c0, w)..]) with cols c0+3m, plus odd rows [(m, [(col, w)..])]."""
    B = _resize_matrix(72, 96) @ _resize_matrix(96, 72)
    plan = {}
    for r in range(4):
        rows = {}
        for m in range(24):
            j = 4 * m + r
            cols = np.nonzero(np.abs(B[j]) > 1e-12)[0]
            rows[m] = [(int(c), float(B[j, c])) for c in cols]
        # find the longest run of m where (cols - 3m, weights) identical
        def key(m):
            return tuple((c - 3 * m, round(w, 9)) for c, w in rows[m])
        best = None
        m = 0
        while m < 24:
            k = key(m)
            m2 = m
            while m2 + 1 < 24 and key(m2 + 1) == k:
                m2 += 1
            if best is None or m2 - m > best[1] - best[0]:
                best = (m, m2)
            m = m2 + 1
        m0, m1 = best
        taps = [(c0 + 3 * m0, w) for c0, w in key(m0)]
        odd = [(m, rows[m]) for m in range(24) if not (m0 <= m <= m1)]
        # verify reconstruction
        for m in range(m0, m1 + 1):
            want = dict(rows[m])
            got = {c0 + 3 * (m - m0): w for c0, w in taps}
            assert set(got) == set(want) and all(
                abs(got[c] - want[c]) < 1e-9 for c in got), (r, m)
        plan[r] = dict(m0=m0, cnt=m1 - m0 + 1, taps=taps, odd=odd)
    return plan


SI2PLAN = _si2_plan()
# slot table for si2 weights (slot idx within si2 region, offset 24)
_SI2_SLOTS = {}
for r in range(4):
    for _, w in SI2PLAN[r]["taps"]:
        _SI2_SLOTS.setdefault(round(w, 9), len(_SI2_SLOTS))
    for _, cws in SI2PLAN[r]["odd"]:
        for _, w in cws:
            _SI2_SLOTS.setdefault(round(w, 9), len(_SI2_SLOTS))


# ---------------- program ----------------
@functools.lru_cache(maxsize=1)
def _build():
    nc = bacc.Bacc("TRN2", target_bir_lowering=False, debug=False)
    dec_d = nc.dram_tensor("dec", [BPC, C, H, W], F32, kind="ExternalInput")
    skip_d = nc.dram_tensor("skip", [BPC, C, H, W], F32, kind="ExternalInput")
    aw_d = nc.dram_tensor("aw", [96, NSLOT, 96], BF16, kind="ExternalInput")
    mw_d = nc.dram_tensor("mw", [128, 12, 128], BF16, kind="ExternalInput")
    coef_d = nc.dram_tensor("coef", [128, 8], F32, kind="ExternalInput")
    out_d = nc.dram_tensor("out", [BPC, C, H, W], F32, kind="ExternalOutput")

    with tile.TileContext(nc) as tc:
        with (
            tc.tile_pool(name="const", bufs=1) as pconst,
            tc.tile_pool(name="ystage", bufs=2) as pystage,
            tc.tile_pool(name="yfull", bufs=1) as pyfull,
            tc.tile_pool(name="pyw", bufs=1) as pyw,
            tc.tile_pool(name="phd", bufs=1) as phd,
            tc.tile_pool(name="pslab", bufs=2) as pslab,
            tc.tile_pool(name="pdl", bufs=1) as pdl,
            tc.tile_pool(name="pwcon", bufs=1) as pwcon,
            tc.tile_pool(name="pdog", bufs=2) as pdog,
            tc.tile_pool(name="pmixs", bufs=2) as pmixs,
            tc.tile_pool(name="ps_tr", bufs=1, space="PSUM") as ps_tr,
            tc.tile_pool(name="ps_d", bufs=2, space="PSUM") as ps_d,
            tc.tile_pool(name="ps_wed", bufs=1, space="PSUM") as ps_wed,
            tc.tile_pool(name="ps_mix", bufs=2, space="PSUM") as ps_mix,
        ):
            aw_sb = pconst.tile([96, NSLOT, 96], BF16)
            nc.sync.dma_start(out=aw_sb[:], in_=aw_d[:])
            mw_sb = pconst.tile([128, 12, 128], BF16)
            nc.sync.dma_start(out=mw_sb[:], in_=mw_d[:])
            coef_sb = pconst.tile([128, 8], F32)
            nc.sync.dma_start(out=coef_sb[:], in_=coef_d[:])
            ident = pconst.tile([128, 128], BF16)
            make_identity(nc, ident[:])

            def cf(cb, j):
                return coef_sb[:, cb * 4 + j : cb * 4 + j + 1]

            def emit_y(s, cb):
                """y16 [128,96,96] bf16 via DMA + gpsimd accum-DMA + DVE cast."""
                csl = slice(cb * 128, (cb + 1) * 128)
                y16 = pyfull.tile([128, H, W], BF16, tag=f"y{cb}", name=f"y_{s}_{cb}")
                for st in range(24):
                    rsl = slice(st * 4, st * 4 + 4)
                    d32 = pystage.tile([128, 4, W], F32, tag="d32",
                                       name=f"d32_{s}_{cb}_{st}")
                    nc.sync.dma_start(out=d32[:], in_=dec_d[s, csl, rsl])
                    k32 = pystage.tile([128, 4, W], F32, tag="k32",
                                       name=f"k32_{s}_{cb}_{st}")
                    nc.sync.dma_start(out=k32[:], in_=skip_d[s, csl, rsl])
                    nc.vector.tensor_add(out=y16[:, rsl, :], in0=d32[:], in1=k32[:])
                return y16

            def emit_transpose(s, cb, y16):
                yw = pyw.tile([96, H, 128], BF16, tag="yw", name=f"yw_{s}_{cb}")
                for g in range(12):
                    pt = ps_tr.tile([96, 8, 128], BF16, tag="tr")
                    for k in range(8):
                        nc.tensor.transpose(pt[:, k, :], y16[:, g * 8 + k, :],
                                            ident[:])
                    nc.vector.tensor_copy(out=yw[:, g * 8 : g * 8 + 8, :], in_=pt[:])
                return yw

            def emit_hd(s, cb, yw):
                hds = []
                for si in range(3):
                    hd = phd.tile([96, NS[si], 128], BF16, tag=f"hd{si}",
                                  name=f"hd{si}_{s}_{cb}")
                    if si == 0:
                        nc.vector.tensor_add(out=hd[:], in0=yw[:, _sl(1, 4, 24), :],
                                             in1=yw[:, _sl(2, 4, 24), :])
                    elif si == 1:
                        nc.vector.tensor_add(out=hd[:], in0=yw[:, _sl(0, 2, 48), :],
                                             in1=yw[:, _sl(1, 2, 48), :])
                    else:
                        for q, f in ((0, 1 / 6), (1, 0.5), (2, 5 / 6)):
                            a = yw[:, _sl(q, 4, 24), :]
                            b = yw[:, _sl(q + 1, 4, 24), :]
                            if q == 1:
                                nc.vector.tensor_add(out=hd[:, _sl(1, 3, 24), :],
                                                     in0=a, in1=b)
                            else:
                                for hh in range(2):
                                    qsl = slice(hh * 12, hh * 12 + 12)
                                    dl = pdl.tile([96, 12, 128], BF16, tag="dl75")
                                    nc.vector.tensor_sub(
                                        out=dl[:], in0=b[:, qsl, :], in1=a[:, qsl, :])
                                    nc.vector._custom_dve(
                                        AFFINE_THEN_ADD,
                                        out=hd[:, _sl(q + 36 * hh, 3, 12), :],
                                        in0=dl[:], in1=a[:, qsl, :],
                                        s0=float(f), s1=0.0)
                    hds.append(hd)
                return hds

            def emit_fea(s, cb, yw, hds, wcon):
                cp_by_j = [dict(UP[si][1]) for si in range(3)]
                for r in range(4):
                    for g in range(3):
                        mlo = g * 8
                        es = []
                        for si in range(3):
                            pd = ps_d.tile([96, 8, 128], F32, tag="d")
                            runs, _ = UP[si]
                            for hb in range(2):
                                blo = mlo + hb * 4
                                psl = slice(hb * 4, hb * 4 + 4)
                                nc.tensor.matmul(
                                    pd[:, psl, :], lhsT=ident[0:96, 0:96],
                                    rhs=yw[:, _sl(4 * blo + r, 4, 4), :],
                                    start=True, stop=False)
                                mms = []
                                if r in runs:
                                    m0, cnt, a0, S, f = runs[r]
                                    ilo = max(blo, m0)
                                    ihi = min(blo + 4, m0 + cnt)
                                    if ihi > ilo:
                                        n = ihi - ilo
                                        av = a0 + S * (ilo - m0)
                                        osl = pd[:, ilo - mlo : ihi - mlo, :]
                                        base = si * 10 + 2 * r
                                        mms.append((osl, base,
                                                    hds[si][:, _sl(av, S, n), :]))
                                        mms.append((osl, base + 1,
                                                    hds[si][:, _sl(av + 1, S, n), :]))
                                for m in range(blo, blo + 4):
                                    j = 4 * m + r
                                    if j in cp_by_j[si]:
                                        src = cp_by_j[si][j]
                                        slot = si * 10 + 8
                                        if si == 2 and src % 3 == 1:
                                            slot = si * 10 + 9
                                        mms.append((pd[:, m - mlo, :], slot,
                                                    hds[si][:, src, :]))
                                for i, (osl, slot, rhs) in enumerate(mms):
                                    nc.tensor.matmul(
                                        osl, lhsT=aw_sb[:, slot, :], rhs=rhs,
                                        start=False, stop=(i == len(mms) - 1))
                            e = pslab.tile([96, 8, 128], BF16, tag=f"e{si}")
                            nc.scalar.activation(e[:], pd[:], AF.Abs)
                            es.append(e)
                        mx = pslab.tile([96, 8, 128], BF16, tag="mx")
                        nc.vector.tensor_tensor(out=mx[:], in0=es[0][:],
                                                in1=es[1][:], op=AL.max)
                        nc.vector.tensor_tensor(out=mx[:], in0=mx[:],
                                                in1=es[2][:], op=AL.max)
                        mn = pslab.tile([96, 8, 128], BF16, tag="mn")
                        nc.vector.tensor_tensor(out=mn[:], in0=es[0][:],
                                                in1=es[1][:], op=AL.min)
                        nc.vector.tensor_tensor(out=mn[:], in0=mn[:],
                                                in1=es[2][:], op=AL.min)
                        wg = pslab.tile([96, 8, 128], BF16, tag="mx")
                        nc.vector.tensor_sub(out=wg[:], in0=mx[:], in1=mn[:])
                        pw = ps_wed.tile([128, 8, 96], BF16, tag="wed")
                        for k in range(8):
                            nc.tensor.transpose(pw[:, k, :], wg[:, k, :],
                                                ident[0:96, 0:96])
                        nc.vector.tensor_copy(
                            out=wcon[:, _sl(4 * mlo + r, 4, 8), :], in_=pw[:])

            def emit_dog_mix(s, y16s, wcons):
                out_f = [out_d[s, slice(ob * 128, (ob + 1) * 128)]
                         .rearrange("c h w -> c (h w)") for ob in range(2)]
                skip_f = [skip_d[s, slice(ob * 128, (ob + 1) * 128)]
                          .rearrange("c h w -> c (h w)") for ob in range(2)]
                for st in range(12):
                    h0 = st * 8
                    hsl = slice(h0, h0 + 8)
                    r0, r1 = max(h0 - 1, 0), min(h0 + 9, H)
                    zsts = []
                    for cb in range(2):
                        y16, wcon = y16s[cb], wcons[cb]
                        vh = pdog.tile([128, 8, W], BF16, tag="vh")
                        g0, g1 = max(h0, 1), min(h0 + 8, 95)
                        nc.vector.tensor_add(
                            out=vh[:, g0 - h0 : g1 - h0, :],
                            in0=y16[:, g0 - 1 : g1 - 1, :],
                            in1=y16[:, g0 + 1 : g1 + 1, :])
                        if h0 == 0:
                            nc.vector.tensor_copy(out=vh[:, 0, :], in_=y16[:, 1, :])
                        if h0 + 8 == H:
                            nc.vector.tensor_copy(out=vh[:, 7, :],
                                                  in_=y16[:, 94, :])
                        ctr = slice(0, 8)
                        # u stripe (8 rows): S_w y
                        ust = pdog.tile([128, 8, W], BF16, tag="ust")
                        nc.gpsimd.tensor_tensor(out=ust[:, :, 1:95],
                                                in0=y16[:, hsl, 0:94],
                                                in1=y16[:, hsl, 2:96], op=AL.add)
                        nc.gpsimd.tensor_copy(out=ust[:, :, 0],
                                              in_=y16[:, hsl, 1])
                        nc.gpsimd.tensor_copy(out=ust[:, :, 95],
                                              in_=y16[:, hsl, 94])
                        # c1 = u + v (gpsimd)
                        c1 = pdog.tile([128, 8, W], BF16, tag="c1")
                        nc.gpsimd.tensor_tensor(out=c1[:], in0=ust[:],
                                                in1=vh[:, ctr, :], op=AL.add)
                        # Q = S_w v (center rows)
                        qt = pdog.tile([128, 8, W], BF16, tag="qt")
                        nc.vector.tensor_add(out=qt[:, :, 1:95],
                                             in0=vh[:, ctr, 0:94],
                                             in1=vh[:, ctr, 2:96])
                        nc.vector.tensor_copy(out=qt[:, :, 0], in_=vh[:, ctr, 1])
                        nc.vector.tensor_copy(out=qt[:, :, 95],
                                              in_=vh[:, ctr, 94])
                        t2 = pdog.tile([128, 8, W], BF16, tag="t2")
                        nc.scalar.activation(t2[:], c1[:], AF.Copy,
                                             scale=cf(cb, 1))
                        zst = pdog.tile([128, 8, W], BF16, tag=f"zst{cb}")
                        nc.vector._custom_dve(AFFINE_THEN_ADD, out=zst[:],
                                              in0=qt[:], in1=t2[:],
                                              s0=cf(cb, 2), s1=0.0)
                        zsts.append(zst)
                    z0f = zsts[0][:].rearrange("c h w -> c (h w)")
                    z1f = zsts[1][:].rearrange("c h w -> c (h w)")
                    w0f = wcons[0][:].rearrange("c h w -> c (h w)")
                    w1f = wcons[1][:].rearrange("c h w -> c (h w)")
                    y0f = y16s[0][:].rearrange("c h w -> c (h w)")
                    y1f = y16s[1][:].rearrange("c h w -> c (h w)")
                    for ob in range(2):
                        col0 = h0 * W
                        csl = slice(col0, col0 + 768)
                        sk32 = pmixs.tile([128, 768], F32, tag="sk32")
                        nc.sync.dma_start(out=sk32[:], in_=skip_f[ob][:, csl])
                        sk = pmixs.tile([128, 768], BF16, tag="sk")
                        nc.vector.tensor_copy(out=sk[:], in_=sk32[:])
                        ot = pmixs.tile([128, 768], F32, tag="ot")
                        for hf in range(2):
                            lsl = slice(hf * 384, hf * 384 + 384)
                            pmx = ps_mix.tile([128, 384], F32, tag="pmix")
                            nc.tensor.matmul(pmx[:], lhsT=ident[:],
                                             rhs=sk[:, lsl],
                                             start=True, stop=False)
                            nc.tensor.matmul(pmx[:], lhsT=mw_sb[:, ob, :],
                                             rhs=z0f[:, lsl],
                                             start=False, stop=False)
                            nc.tensor.matmul(pmx[:], lhsT=mw_sb[:, 2 + ob, :],
                                             rhs=z1f[:, lsl],
                                             start=False, stop=False)
                            nc.tensor.matmul(pmx[:], lhsT=mw_sb[:, 4 + ob, :],
                                             rhs=w0f[:, csl][:, lsl],
                                             start=False, stop=False)
                            nc.tensor.matmul(pmx[:], lhsT=mw_sb[:, 6 + ob, :],
                                             rhs=w1f[:, csl][:, lsl],
                                             start=False, stop=False)
                            nc.tensor.matmul(pmx[:], lhsT=mw_sb[:, 8 + ob, :],
                                             rhs=y0f[:, csl][:, lsl],
                                             start=False, stop=False)
                            nc.tensor.matmul(pmx[:], lhsT=mw_sb[:, 10 + ob, :],
                                             rhs=y1f[:, csl][:, lsl],
                                             start=False, stop=True)
                            nc.scalar.copy(ot[:, lsl], pmx[:])
                        nc.scalar.dma_start(out=out_f[ob][:, csl], in_=ot[:])

            for s in range(BPC):
                y16s, wcons = {}, {}
                for cb in range(2):
                    y16 = emit_y(s, cb)
                    y16s[cb] = y16
                    yw = emit_transpose(s, cb, y16)
                    hds = emit_hd(s, cb, yw)
                    wcon = pwcon.tile([128, H, W], BF16, tag=f"wc{cb}",
                                      name=f"wc_{s}_{cb}")
                    wcons[cb] = wcon
                    emit_fea(s, cb, yw, hds, wcon)
                emit_dog_mix(s, y16s, wcons)
    nc.finalize()
    return nc


# ---------------- host consts ----------------
def _consts(w_fea, sigma1, sigma2, mixer_w):
    wf = w_fea.reshape(C).astype(np.float64) * (2.0 / 3.0)
    t, n = {}, {}
    for i, sg in ((1, sigma1), (2, sigma2)):
        sig = 2.0 / (1.0 + np.exp(-sg.reshape(C).astype(np.float64)))
        t[i] = np.exp(-1.0 / (2.0 * sig ** 2))
        n[i] = (1.0 + 2.0 * t[i]) ** -2
    alpha = n[1] - n[2]
    beta = n[1] * t[1] - n[2] * t[2]
    gamma = n[1] * t[1] ** 2 - n[2] * t[2] ** 2

    coef = np.zeros((128, 8), dtype=np.float32)
    for cb in range(2):
        ch = slice(cb * 128, (cb + 1) * 128)
        coef[:, cb * 4 + 0] = wf[ch]
        coef[:, cb * 4 + 1] = beta[ch]
        coef[:, cb * 4 + 2] = gamma[ch]
        coef[:, cb * 4 + 3] = 3.0 + alpha[ch]

    aw = np.zeros((96, NSLOT, 96), dtype=np.float64)
    for si in range(3):
        ns = NS[si]
        A = _resize_matrix(ns, H) @ _resize_matrix(H, ns)
        fold = 0.5 if si in (0, 1) else 1.0
        Af = (fold * A).T
        runs, _ = _up_runs(ns)
        base = si * 10
        for r, (m0, cnt, a0, S, f) in runs.items():
            if si == 2:
                wa = (1.0 - f) * _si2_scale(a0)
                wb = f * _si2_scale(a0 + 1)
            else:
                wa, wb = (1.0 - f), f
            aw[:, base + 2 * r, :] = -wa * Af
            aw[:, base + 2 * r + 1, :] = -wb * Af
        aw[:, base + 8, :] = -Af
        aw[:, base + 9, :] = -0.5 * Af
    aw = aw.astype(ml_dtypes.bfloat16)

    M = mixer_w.reshape(C, C).astype(np.float64)
    mw = np.zeros((128, 12, 128), dtype=np.float64)
    a3 = 3.0 + alpha
    for kc in range(2):
        for ob in range(2):
            blk = M[ob * 128:(ob + 1) * 128, kc * 128:(kc + 1) * 128].T
            mw[:, kc * 2 + ob, :] = blk
            mw[:, 4 + kc * 2 + ob, :] = wf[kc * 128:(kc + 1) * 128, None] * blk
            mw[:, 8 + kc * 2 + ob, :] = a3[kc * 128:(kc + 1) * 128, None] * blk
    mw = mw.astype(ml_dtypes.bfloat16)
    return aw, mw, coef


def kernel(skip, dec, w_fea, sigma1, sigma2, mixer_w, _trace=[False]):
    skip = np.ascontiguousarray(np.asarray(skip, dtype=np.float32))
    dec = np.ascontiguousarray(np.asarray(dec, dtype=np.float32))
    aw, mw, coef = _consts(
        np.asarray(w_fea), np.asarray(sigma1), np.asarray(sigma2),
        np.asarray(mixer_w))
    nc = _build()
    in_maps = []
    for i in range(NCORES):
        in_maps.append({
            "dec": dec[BPC * i : BPC * (i + 1)],
            "skip": skip[BPC * i : BPC * (i + 1)],
            "aw": aw, "mw": mw, "coef": coef,
        })
    res = run_bass_kernel_spmd(nc, in_maps, core_ids=list(range(NCORES)),
                               trace=_trace[0])
    kernel.last_result = res
    return np.concatenate([r["out"] for r in res.results], axis=0)


kernel.last_result = None


# revision 6
# speedup vs baseline: 1.3664x; 1.0246x over previous
"""Trainium2 Bass kernel v2 for nn_DSEBlock — engine-balanced redesign.

Decomposition (validated in mirror.py vs reference):
  y = dec + skip                         (DMA-accumulated, fp32, cast bf16)
  FEA: per scale si: psum = y - U_si(hd_si) x A_w,si  (PE, planar 4-row banks)
       e_si = |psum| (ACT abs) ; wedge = max3(e) - min3(e) (DVE/GP slab TT)
       wcon[c,h,w] = XBAR-DMA row transposes of wedge slabs
  DoG: u = S_w y (DVE), v = S_h y (GP), c1 = v + u (gpsimd accum-DMA),
       Q = S_h u (DVE)
  z   = (3+a)y + b*c1 + g*Q + wf'*wcon   (AFFINE_THEN_ADD chain + ACT)
  out = M z + skip                       (PE mixer, DVE psum+skip extract)

Engines: PE ~250us, DVE ~300, ACT ~280, GP ~240, DMA ~270 per core (est).
"""
import functools

import ml_dtypes
import numpy as np

import concourse.bass as bass
import concourse.mybir as mybir
import concourse.tile as tile
from concourse import bacc
from concourse.bass import ts
from concourse.bass_utils import run_bass_kernel_spmd
from concourse.dve_ops import AFFINE_THEN_ADD
from concourse.masks import make_identity

F32 = mybir.dt.float32
BF16 = mybir.dt.bfloat16
AL = mybir.AluOpType
AF = mybir.ActivationFunctionType

B, C, H, W = 16, 256, 96, 96
NCORES = 8
BPC = B // NCORES
SCALES = [0.25, 0.5, 0.75]
NS = [24, 48, 72]
HW = H * W


def _sl(start, step, cnt):
    return slice(start, start + step * (cnt - 1) + 1, step)


# ---------------- host-side resize structure ----------------
def _resize_matrix(n_in, n_out):
    A = np.zeros((n_out, n_in), dtype=np.float64)
    scale = n_in / n_out
    for j in range(n_out):
        x = (j + 0.5) * scale - 0.5
        i0 = int(np.floor(x))
        f = x - i0
        A[j, min(max(i0, 0), n_in - 1)] += 1.0 - f
        A[j, min(max(i0 + 1, 0), n_in - 1)] += f
    return A


def _up_runs(ns):
    scale = ns / H
    S = {24: 1, 48: 2, 72: 3}[ns]
    groups, copies = {}, []
    for j in range(H):
        x = (j + 0.5) * scale - 0.5
        i0 = int(np.floor(x))
        f = x - i0
        if i0 < 0:
            copies.append((j, 0))
            continue
        if i0 + 1 > ns - 1:
            copies.append((j, ns - 1))
            continue
        groups.setdefault(j % 4, []).append((j // 4, i0, f))
    runs = {}
    for r, items in groups.items():
        items.sort()
        ms = [m for m, _, _ in items]
        assert ms == list(range(ms[0], ms[-1] + 1))
        runs[r] = (ms[0], len(ms), items[0][1], S, items[0][2])
    return runs, copies


# aw slot map: per si: base = si*12: slots base+2r (tap a), base+2r+1 (tap b),
# base+8: clamp full, base+9: clamp half (si2 residue-1 sources)
NSLOT = 30


def _ywrows(yw, start, step, cnt):
    return yw[:, _sl(start, step, cnt), :]


def _si2_scale(a):
    return 0.5 if a % 3 == 1 else 1.0


UP = [_up_runs(ns) for ns in NS]


def _si2_plan():
    """B75 = U75@D75 folded tap plan: per r: interior run (m0, cnt,
    taps=[(c0, w)..]) with cols c0+3m, plus odd rows [(m, [(col, w)..])]."""
    B = _resize_matrix(72, 96) @ _resize_matrix(96, 72)
    plan = {}
    for r in range(4):
        rows = {}
        for m in range(24):
            j = 4 * m + r
            cols = np.nonzero(np.abs(B[j]) > 1e-12)[0]
            rows[m] = [(int(c), float(B[j, c])) for c in cols]
        # find the longest run of m where (cols - 3m, weights) identical
        def key(m):
            return tuple((c - 3 * m, round(w, 9)) for c, w in rows[m])
        best = None
        m = 0
        while m < 24:
            k = key(m)
            m2 = m
            while m2 + 1 < 24 and key(m2 + 1) == k:
                m2 += 1
            if best is None or m2 - m > best[1] - best[0]:
                best = (m, m2)
            m = m2 + 1
        m0, m1 = best
        taps = [(c0 + 3 * m0, w) for c0, w in key(m0)]
        odd = [(m, rows[m]) for m in range(24) if not (m0 <= m <= m1)]
        # verify reconstruction
        for m in range(m0, m1 + 1):
            want = dict(rows[m])
            got = {c0 + 3 * (m - m0): w for c0, w in taps}
            assert set(got) == set(want) and all(
                abs(got[c] - want[c]) < 1e-9 for c in got), (r, m)
        plan[r] = dict(m0=m0, cnt=m1 - m0 + 1, taps=taps, odd=odd)
    return plan


SI2PLAN = _si2_plan()
# slot table for si2 weights (slot idx within si2 region, offset 24)
_SI2_SLOTS = {}
for r in range(4):
    for _, w in SI2PLAN[r]["taps"]:
        _SI2_SLOTS.setdefault(round(w, 9), len(_SI2_SLOTS))
    for _, cws in SI2PLAN[r]["odd"]:
        for _, w in cws:
            _SI2_SLOTS.setdefault(round(w, 9), len(_SI2_SLOTS))


# ---------------- program ----------------
@functools.lru_cache(maxsize=1)
def _build():
    nc = bacc.Bacc("TRN2", target_bir_lowering=False, debug=False)
    dec_d = nc.dram_tensor("dec", [BPC, C, H, W], F32, kind="ExternalInput")
    skip_d = nc.dram_tensor("skip", [BPC, C, H, W], F32, kind="ExternalInput")
    aw_d = nc.dram_tensor("aw", [96, NSLOT, 96], BF16, kind="ExternalInput")
    mw_d = nc.dram_tensor("mw", [128, 12, 128], BF16, kind="ExternalInput")
    coef_d = nc.dram_tensor("coef", [128, 8], F32, kind="ExternalInput")
    out_d = nc.dram_tensor("out", [BPC, C, H, W], F32, kind="ExternalOutput")

    with tile.TileContext(nc) as tc:
        with (
            tc.tile_pool(name="const", bufs=1) as pconst,
            tc.tile_pool(name="ystage", bufs=3) as pystage,
            tc.tile_pool(name="yfull", bufs=1) as pyfull,
            tc.tile_pool(name="pyw", bufs=1) as pyw,
            tc.tile_pool(name="phd", bufs=1) as phd,
            tc.tile_pool(name="pslab", bufs=2) as pslab,
            tc.tile_pool(name="pdl", bufs=1) as pdl,
            tc.tile_pool(name="pwcon", bufs=1) as pwcon,
            tc.tile_pool(name="pdog", bufs=2) as pdog,
            tc.tile_pool(name="pmixs", bufs=2) as pmixs,
            tc.tile_pool(name="ps_tr", bufs=1, space="PSUM") as ps_tr,
            tc.tile_pool(name="ps_d", bufs=2, space="PSUM") as ps_d,
            tc.tile_pool(name="ps_wed", bufs=1, space="PSUM") as ps_wed,
            tc.tile_pool(name="ps_mix", bufs=2, space="PSUM") as ps_mix,
        ):
            aw_sb = pconst.tile([96, NSLOT, 96], BF16)
            nc.sync.dma_start(out=aw_sb[:], in_=aw_d[:])
            mw_sb = pconst.tile([128, 12, 128], BF16)
            nc.sync.dma_start(out=mw_sb[:], in_=mw_d[:])
            coef_sb = pconst.tile([128, 8], F32)
            nc.sync.dma_start(out=coef_sb[:], in_=coef_d[:])
            ident = pconst.tile([128, 128], BF16)
            make_identity(nc, ident[:])

            def cf(cb, j):
                return coef_sb[:, cb * 4 + j : cb * 4 + j + 1]

            def emit_y(s, cb):
                """y16 [128,96,96] bf16 via DMA + gpsimd accum-DMA + DVE cast."""
                csl = slice(cb * 128, (cb + 1) * 128)
                y16 = pyfull.tile([128, H, W], BF16, tag=f"y{cb}", name=f"y_{s}_{cb}")
                for st in range(24):
                    rsl = slice(st * 4, st * 4 + 4)
                    d32 = pystage.tile([128, 4, W], F32, tag="d32",
                                       name=f"d32_{s}_{cb}_{st}")
                    nc.sync.dma_start(out=d32[:], in_=dec_d[s, csl, rsl])
                    k32 = pystage.tile([128, 4, W], F32, tag="k32",
                                       name=f"k32_{s}_{cb}_{st}")
                    nc.sync.dma_start(out=k32[:], in_=skip_d[s, csl, rsl])
                    nc.vector.tensor_add(out=y16[:, rsl, :], in0=d32[:], in1=k32[:])
                return y16

            def emit_transpose(s, cb, y16):
                yw = pyw.tile([96, H, 128], BF16, tag="yw", name=f"yw_{s}_{cb}")
                for g in range(12):
                    pt = ps_tr.tile([96, 8, 128], BF16, tag="tr")
                    for k in range(8):
                        nc.tensor.transpose(pt[:, k, :], y16[:, g * 8 + k, :],
                                            ident[:])
                    nc.vector.tensor_copy(out=yw[:, g * 8 : g * 8 + 8, :], in_=pt[:])
                return yw

            def emit_hd(s, cb, yw):
                hds = []
                for si in range(3):
                    hd = phd.tile([96, NS[si], 128], BF16, tag=f"hd{si}",
                                  name=f"hd{si}_{s}_{cb}")
                    if si == 0:
                        nc.vector.tensor_add(out=hd[:], in0=yw[:, _sl(1, 4, 24), :],
                                             in1=yw[:, _sl(2, 4, 24), :])
                    elif si == 1:
                        nc.vector.tensor_add(out=hd[:], in0=yw[:, _sl(0, 2, 48), :],
                                             in1=yw[:, _sl(1, 2, 48), :])
                    else:
                        for q, f in ((0, 1 / 6), (1, 0.5), (2, 5 / 6)):
                            a = yw[:, _sl(q, 4, 24), :]
                            b = yw[:, _sl(q + 1, 4, 24), :]
                            if q == 1:
                                nc.vector.tensor_add(out=hd[:, _sl(1, 3, 24), :],
                                                     in0=a, in1=b)
                            else:
                                for hh in range(2):
                                    qsl = slice(hh * 12, hh * 12 + 12)
                                    dl = pdl.tile([96, 12, 128], BF16, tag="dl75")
                                    nc.vector.tensor_sub(
                                        out=dl[:], in0=b[:, qsl, :], in1=a[:, qsl, :])
                                    nc.vector._custom_dve(
                                        AFFINE_THEN_ADD,
                                        out=hd[:, _sl(q + 36 * hh, 3, 12), :],
                                        in0=dl[:], in1=a[:, qsl, :],
                                        s0=float(f), s1=0.0)
                    hds.append(hd)
                return hds

            def emit_fea(s, cb, yw, hds, wcon):
                cp_by_j = [dict(UP[si][1]) for si in range(3)]
                for r in range(4):
                    for g in range(3):
                        mlo = g * 8
                        es = []
                        for si in range(3):
                            pd = ps_d.tile([96, 8, 128], F32, tag="d")
                            runs, _ = UP[si]
                            for hb in range(2):
                                blo = mlo + hb * 4
                                psl = slice(hb * 4, hb * 4 + 4)
                                nc.tensor.matmul(
                                    pd[:, psl, :], lhsT=ident[0:96, 0:96],
                                    rhs=yw[:, _sl(4 * blo + r, 4, 4), :],
                                    start=True, stop=False)
                                mms = []
                                if r in runs:
                                    m0, cnt, a0, S, f = runs[r]
                                    ilo = max(blo, m0)
                                    ihi = min(blo + 4, m0 + cnt)
                                    if ihi > ilo:
                                        n = ihi - ilo
                                        av = a0 + S * (ilo - m0)
                                        osl = pd[:, ilo - mlo : ihi - mlo, :]
                                        base = si * 10 + 2 * r
                                        mms.append((osl, base,
                                                    hds[si][:, _sl(av, S, n), :]))
                                        mms.append((osl, base + 1,
                                                    hds[si][:, _sl(av + 1, S, n), :]))
                                for m in range(blo, blo + 4):
                                    j = 4 * m + r
                                    if j in cp_by_j[si]:
                                        src = cp_by_j[si][j]
                                        slot = si * 10 + 8
                                        if si == 2 and src % 3 == 1:
                                            slot = si * 10 + 9
                                        mms.append((pd[:, m - mlo, :], slot,
                                                    hds[si][:, src, :]))
                                for i, (osl, slot, rhs) in enumerate(mms):
                                    nc.tensor.matmul(
                                        osl, lhsT=aw_sb[:, slot, :], rhs=rhs,
                                        start=False, stop=(i == len(mms) - 1))
                            e = pslab.tile([96, 8, 128], BF16, tag=f"e{si}")
                            nc.scalar.activation(e[:], pd[:], AF.Abs)
                            es.append(e)
                        mx = pslab.tile([96, 8, 128], BF16, tag="mx")
                        nc.vector.tensor_tensor(out=mx[:], in0=es[0][:],
                                                in1=es[1][:], op=AL.max)
                        nc.vector.tensor_tensor(out=mx[:], in0=mx[:],
                                                in1=es[2][:], op=AL.max)
                        mn = pslab.tile([96, 8, 128], BF16, tag="mn")
                        nc.vector.tensor_tensor(out=mn[:], in0=es[0][:],
                                                in1=es[1][:], op=AL.min)
                        nc.vector.tensor_tensor(out=mn[:], in0=mn[:],
                                                in1=es[2][:], op=AL.min)
                        wg = pslab.tile([96, 8, 128], BF16, tag="mx")
                        nc.vector.tensor_sub(out=wg[:], in0=mx[:], in1=mn[:])
                        pw = ps_wed.tile([128, 8, 96], BF16, tag="wed")
                        for k in range(8):
                            nc.tensor.transpose(pw[:, k, :], wg[:, k, :],
                                                ident[0:96, 0:96])
                        nc.vector.tensor_copy(
                            out=wcon[:, _sl(4 * mlo + r, 4, 8), :], in_=pw[:])

            def emit_dog_mix(s, y16s, wcons):
                out_f = [out_d[s, slice(ob * 128, (ob + 1) * 128)]
                         .rearrange("c h w -> c (h w)") for ob in range(2)]
                skip_f = [skip_d[s, slice(ob * 128, (ob + 1) * 128)]
                          .rearrange("c h w -> c (h w)") for ob in range(2)]
                for st in range(12):
                    h0 = st * 8
                    hsl = slice(h0, h0 + 8)
                    r0, r1 = max(h0 - 1, 0), min(h0 + 9, H)
                    zsts = []
                    for cb in range(2):
                        y16, wcon = y16s[cb], wcons[cb]
                        vh = pdog.tile([128, 8, W], BF16, tag="vh")
                        g0, g1 = max(h0, 1), min(h0 + 8, 95)
                        nc.vector.tensor_add(
                            out=vh[:, g0 - h0 : g1 - h0, :],
                            in0=y16[:, g0 - 1 : g1 - 1, :],
                            in1=y16[:, g0 + 1 : g1 + 1, :])
                        if h0 == 0:
                            nc.vector.tensor_copy(out=vh[:, 0, :], in_=y16[:, 1, :])
                        if h0 + 8 == H:
                            nc.vector.tensor_copy(out=vh[:, 7, :],
                                                  in_=y16[:, 94, :])
                        ctr = slice(0, 8)
                        # u stripe (8 rows): S_w y
                        ust = pdog.tile([128, 8, W], BF16, tag="ust")
                        nc.gpsimd.tensor_tensor(out=ust[:, :, 1:95],
                                                in0=y16[:, hsl, 0:94],
                                                in1=y16[:, hsl, 2:96], op=AL.add)
                        nc.gpsimd.tensor_copy(out=ust[:, :, 0],
                                              in_=y16[:, hsl, 1])
                        nc.gpsimd.tensor_copy(out=ust[:, :, 95],
                                              in_=y16[:, hsl, 94])
                        # c1 = u + v (gpsimd)
                        c1 = pdog.tile([128, 8, W], BF16, tag="c1")
                        nc.gpsimd.tensor_tensor(out=c1[:], in0=ust[:],
                                                in1=vh[:, ctr, :], op=AL.add)
                        # Q = S_w v (center rows)
                        qt = pdog.tile([128, 8, W], BF16, tag="qt")
                        nc.vector.tensor_add(out=qt[:, :, 1:95],
                                             in0=vh[:, ctr, 0:94],
                                             in1=vh[:, ctr, 2:96])
                        nc.vector.tensor_copy(out=qt[:, :, 0], in_=vh[:, ctr, 1])
                        nc.vector.tensor_copy(out=qt[:, :, 95],
                                              in_=vh[:, ctr, 94])
                        t2 = pdog.tile([128, 8, W], BF16, tag="t2")
                        nc.scalar.activation(t2[:], c1[:], AF.Copy,
                                             scale=cf(cb, 1))
                        zst = pdog.tile([128, 8, W], BF16, tag=f"zst{cb}")
                        nc.vector._custom_dve(AFFINE_THEN_ADD, out=zst[:],
                                              in0=qt[:], in1=t2[:],
                                              s0=cf(cb, 2), s1=0.0)
                        zsts.append(zst)
                    z0f = zsts[0][:].rearrange("c h w -> c (h w)")
                    z1f = zsts[1][:].rearrange("c h w -> c (h w)")
                    w0f = wcons[0][:].rearrange("c h w -> c (h w)")
                    w1f = wcons[1][:].rearrange("c h w -> c (h w)")
                    y0f = y16s[0][:].rearrange("c h w -> c (h w)")
                    y1f = y16s[1][:].rearrange("c h w -> c (h w)")
                    for ob in range(2):
                        col0 = h0 * W
                        csl = slice(col0, col0 + 768)
                        sk = pmixs.tile([128, 768], BF16, tag="sk")
                        for sh in range(2):
                            shs = slice(col0 + sh * 384, col0 + sh * 384 + 384)
                            sk32 = pmixs.tile([128, 384], F32, tag="sk32")
                            nc.sync.dma_start(out=sk32[:], in_=skip_f[ob][:, shs])
                            nc.vector.tensor_copy(
                                out=sk[:, sh * 384 : sh * 384 + 384], in_=sk32[:])
                        ot = pmixs.tile([128, 768], F32, tag="ot")
                        for hf in range(2):
                            lsl = slice(hf * 384, hf * 384 + 384)
                            pmx = ps_mix.tile([128, 384], F32, tag="pmix")
                            nc.tensor.matmul(pmx[:], lhsT=ident[:],
                                             rhs=sk[:, lsl],
                                             start=True, stop=False)
                            nc.tensor.matmul(pmx[:], lhsT=mw_sb[:, ob, :],
                                             rhs=z0f[:, lsl],
                                             start=False, stop=False)
                            nc.tensor.matmul(pmx[:], lhsT=mw_sb[:, 2 + ob, :],
                                             rhs=z1f[:, lsl],
                                             start=False, stop=False)
                            nc.tensor.matmul(pmx[:], lhsT=mw_sb[:, 4 + ob, :],
                                             rhs=w0f[:, csl][:, lsl],
                                             start=False, stop=False)
                            nc.tensor.matmul(pmx[:], lhsT=mw_sb[:, 6 + ob, :],
                                             rhs=w1f[:, csl][:, lsl],
                                             start=False, stop=False)
                            nc.tensor.matmul(pmx[:], lhsT=mw_sb[:, 8 + ob, :],
                                             rhs=y0f[:, csl][:, lsl],
                                             start=False, stop=False)
                            nc.tensor.matmul(pmx[:], lhsT=mw_sb[:, 10 + ob, :],
                                             rhs=y1f[:, csl][:, lsl],
                                             start=False, stop=True)
                            nc.scalar.copy(ot[:, lsl], pmx[:])
                        nc.scalar.dma_start(out=out_f[ob][:, csl], in_=ot[:])

            for s in range(BPC):
                y16s, wcons = {}, {}
                for cb in range(2):
                    y16 = emit_y(s, cb)
                    y16s[cb] = y16
                    yw = emit_transpose(s, cb, y16)
                    hds = emit_hd(s, cb, yw)
                    wcon = pwcon.tile([128, H, W], BF16, tag=f"wc{cb}",
                                      name=f"wc_{s}_{cb}")
                    wcons[cb] = wcon
                    emit_fea(s, cb, yw, hds, wcon)
                emit_dog_mix(s, y16s, wcons)
    nc.finalize()
    return nc


# ---------------- host consts ----------------
def _consts(w_fea, sigma1, sigma2, mixer_w):
    wf = w_fea.reshape(C).astype(np.float64) * (2.0 / 3.0)
    t, n = {}, {}
    for i, sg in ((1, sigma1), (2, sigma2)):
        sig = 2.0 / (1.0 + np.exp(-sg.reshape(C).astype(np.float64)))
        t[i] = np.exp(-1.0 / (2.0 * sig ** 2))
        n[i] = (1.0 + 2.0 * t[i]) ** -2
    alpha = n[1] - n[2]
    beta = n[1] * t[1] - n[2] * t[2]
    gamma = n[1] * t[1] ** 2 - n[2] * t[2] ** 2

    coef = np.zeros((128, 8), dtype=np.float32)
    for cb in range(2):
        ch = slice(cb * 128, (cb + 1) * 128)
        coef[:, cb * 4 + 0] = wf[ch]
        coef[:, cb * 4 + 1] = beta[ch]
        coef[:, cb * 4 + 2] = gamma[ch]
        coef[:, cb * 4 + 3] = 3.0 + alpha[ch]

    aw = np.zeros((96, NSLOT, 96), dtype=np.float64)
    for si in range(3):
        ns = NS[si]
        A = _resize_matrix(ns, H) @ _resize_matrix(H, ns)
        fold = 0.5 if si in (0, 1) else 1.0
        Af = (fold * A).T
        runs, _ = _up_runs(ns)
        base = si * 10
        for r, (m0, cnt, a0, S, f) in runs.items():
            if si == 2:
                wa = (1.0 - f) * _si2_scale(a0)
                wb = f * _si2_scale(a0 + 1)
            else:
                wa, wb = (1.0 - f), f
            aw[:, base + 2 * r, :] = -wa * Af
            aw[:, base + 2 * r + 1, :] = -wb * Af
        aw[:, base + 8, :] = -Af
        aw[:, base + 9, :] = -0.5 * Af
    aw = aw.astype(ml_dtypes.bfloat16)

    M = mixer_w.reshape(C, C).astype(np.float64)
    mw = np.zeros((128, 12, 128), dtype=np.float64)
    a3 = 3.0 + alpha
    for kc in range(2):
        for ob in range(2):
            blk = M[ob * 128:(ob + 1) * 128, kc * 128:(kc + 1) * 128].T
            mw[:, kc * 2 + ob, :] = blk
            mw[:, 4 + kc * 2 + ob, :] = wf[kc * 128:(kc + 1) * 128, None] * blk
            mw[:, 8 + kc * 2 + ob, :] = a3[kc * 128:(kc + 1) * 128, None] * blk
    mw = mw.astype(ml_dtypes.bfloat16)
    return aw, mw, coef


def kernel(skip, dec, w_fea, sigma1, sigma2, mixer_w, _trace=[False]):
    skip = np.ascontiguousarray(np.asarray(skip, dtype=np.float32))
    dec = np.ascontiguousarray(np.asarray(dec, dtype=np.float32))
    aw, mw, coef = _consts(
        np.asarray(w_fea), np.asarray(sigma1), np.asarray(sigma2),
        np.asarray(mixer_w))
    nc = _build()
    in_maps = []
    for i in range(NCORES):
        in_maps.append({
            "dec": dec[BPC * i : BPC * (i + 1)],
            "skip": skip[BPC * i : BPC * (i + 1)],
            "aw": aw, "mw": mw, "coef": coef,
        })
    res = run_bass_kernel_spmd(nc, in_maps, core_ids=list(range(NCORES)),
                               trace=_trace[0])
    kernel.last_result = res
    return np.concatenate([r["out"] for r in res.results], axis=0)


kernel.last_result = None
